# revision 36
# baseline (speedup 1.0000x reference)
"""GAT (2-layer graph attention) Trainium2 Bass kernel, 8-core SPMD.

Sharding: data-parallel over batch (2) x row-blocks (4) -> 8 cores.
Core c handles batch b=c//4, output rows R=[512*(c%4), 512*(c%4+1)).

Key algebra: with z = s_src[i]+s_tgt[j], the GAT edge weight
exp(leaky_relu(z, 0.2)) = max(exp(z), exp(0.2 z)).  Softmax rows are
invariant to a per-row scale, so dividing row i by exp(s_src[i]) gives
unnormalized weights F[j,i] = adj[i,j] * D[j] * max(W[j], g[i]) with
  W[j] = exp(0.8 s_tgt[j]),  D[j] = exp(0.2 s_tgt[j]),  g[i] = exp(-0.8 s_src[i])
-- no per-element transcendentals.  A ones-column in the stationary
operand makes the softmax denominator fall out as a matmul row.

Host prep does layout only (transpose / cast / slice, no math): adj
arrives pre-transposed per core as bf16, x transposed in f32 (scores)
and bf16 (projection), weights pre-transposed in bf16.  Inner work per
(head, j-tile) is one tensor_scalar (4x DVE mode) + one 4-tile-merged
tensor_tensor feeding the TensorE numerator matmul in bf16.

Layer-1 outputs are exchanged within each batch group of 4 cores via a
single AllGather of (proj2^T | s_tgt2).  Work that does not need the
gather is emitted before the collective (tile serializes around it).
"""

import os
import sys

for _p in ("/opt/trn_rl_repo", "/root/.axon_site/_ro/trn_rl_repo"):
    if os.path.isdir(_p) and _p not in sys.path:
        sys.path.insert(0, _p)

import numpy as np
import ml_dtypes

import concourse.bass as bass
import concourse.bacc as bacc
import concourse.mybir as mybir
from concourse import tile
from concourse.bass_utils import run_bass_kernel_spmd

F32 = mybir.dt.float32
BF16 = mybir.dt.bfloat16
AF = mybir.ActivationFunctionType
ALU = mybir.AluOpType
BF16NP = ml_dtypes.bfloat16

BS, N, FIN = 2, 2048, 128
H1, F1 = 8, 64
RB = 512          # row block per core
NJT = N // 128    # 16 j-tiles
NCORES = 8
GROUPS = [[0, 1, 2, 3], [4, 5, 6, 7]]

# layer-1 inner-loop engine split (per head): 16 TSPs, 4 merged TTs
_TSP_POOL1 = {1, 3, 6, 9, 11, 15}       # 6 TSPs per head on Pool
_TSP_ACT1 = {14}                         # 1 TSP per head on ACT (2-op form)
_TT_POOL1 = {3}                          # TT group 3 on Pool, 0-2 DVE
# layer-2: 16 TSPs (4 on ACT via relu/affine pair), 4 merged TTs
_TSP_POOL2 = {1, 5, 9, 13}
_TSP_ACT2 = {3, 7, 11, 15}
_TT_POOL2 = {1, 3}


def build_nc():
    nc = bacc.Bacc("TRN2", target_bir_lowering=False, debug=False,
                   num_devices=NCORES)

    # ---- per-core DRAM I/O (host gives transposed / bf16 layouts) ----
    d_adjT = nc.declare_dram_parameter("adjT", [N, RB], BF16, isOutput=False)
    d_xT = nc.declare_dram_parameter("xT", [FIN, N], F32, isOutput=False)
    d_xTb = nc.declare_dram_parameter("xTb", [FIN, N], BF16, isOutput=False)
    d_xrT = nc.declare_dram_parameter("xrT", [FIN, RB], F32, isOutput=False)
    d_xrTb = nc.declare_dram_parameter("xrTb", [FIN, RB], BF16, isOutput=False)
    d_w1n = nc.declare_dram_parameter("w1n", [H1 * F1, FIN], F32, isOutput=False)
    d_w1Tb = nc.declare_dram_parameter("w1Tb", [FIN, H1 * F1], BF16,
                                       isOutput=False)
    d_ws1Tb = nc.declare_dram_parameter("ws1Tb", [FIN, H1 * F1], BF16,
                                        isOutput=False)
    d_a1s2 = nc.declare_dram_parameter("a1s2", [128, H1], F32, isOutput=False)
    d_a1t2 = nc.declare_dram_parameter("a1t2", [128, H1], F32, isOutput=False)
    d_b1 = nc.declare_dram_parameter("b1", [H1 * F1], F32, isOutput=False)
    d_w2n = nc.declare_dram_parameter("w2n", [F1, H1 * F1], F32, isOutput=False)
    d_w2Tb = nc.declare_dram_parameter("w2Tb", [H1 * F1, F1], BF16,
                                       isOutput=False)
    d_ws2Tb = nc.declare_dram_parameter("ws2Tb", [H1 * F1, F1], BF16,
                                        isOutput=False)
    d_a2p = nc.declare_dram_parameter("a2p", [F1, 2], F32, isOutput=False)
    d_b2 = nc.declare_dram_parameter("b2", [F1], F32, isOutput=False)
    # output: transposed row-block out^T [64, 512] (host transposes back)
    d_out = nc.declare_dram_parameter("outT", [F1, RB], F32, isOutput=True)

    with tile.TileContext(nc) as tc:
        with (
            tc.tile_pool(name="persist", bufs=1) as P,
            tc.tile_pool(name="work", bufs=4) as WK,
            tc.tile_pool(name="gbp", bufs=3) as GB,
            tc.tile_pool(name="ps", bufs=3, space="PSUM") as PS,
            tc.tile_pool(name="psb", bufs=2, space="PSUM") as PSB,
            tc.tile_pool(name="psnum", bufs=3, space="PSUM") as PSN,
            tc.tile_pool(name="dram", bufs=1, space="DRAM") as DR,
        ):
            # ============ loads (emission order ~= DMA priority) ============
            # big/early transfers all on the otherwise-idle SP queue so the
            # ACT/Pool sequencers stay free for compute
            w1n = P.tile([128, 4, FIN], F32, tag="w1n")
            nc.sync.dma_start(w1n[:], d_w1n.rearrange("(k p) c -> p k c", p=128))
            a1sT = P.tile([128, H1], F32, tag="a1sT")
            nc.scalar.dma_start(a1sT[:], d_a1s2[:, :])
            a1tT = P.tile([128, H1], F32, tag="a1tT")
            nc.gpsimd.dma_start(a1tT[:], d_a1t2[:, :])
            xrT = P.tile([128, RB], F32, tag="xrT")
            nc.sync.dma_start(xrT[:], d_xrT[:, :])

            xT = P.tile([128, N], F32, tag="xT")
            nc.sync.dma_start(xT[:, 0:1024], d_xT[:, 0:1024])
            w1Tb = P.tile([128, H1 * F1], BF16, tag="w1Tb")
            nc.gpsimd.dma_start(w1Tb[:], d_w1Tb[:, :])
            xTb = P.tile([128, N], BF16, tag="xTb")
            nc.scalar.dma_start(xTb[:, 0:1024], d_xTb[:, 0:1024])
            adjT = P.tile([128, NJT, RB], BF16, tag="adjT")
            adjT_src = d_adjT.rearrange("(t p) i -> p t i", p=128)
            nc.sync.dma_start(adjT[:, 0:4, :], adjT_src[:, 0:4, :])
            nc.sync.dma_start(xT[:, 1024:2048], d_xT[:, 1024:2048])
            nc.scalar.dma_start(xTb[:, 1024:2048], d_xTb[:, 1024:2048])
            nc.sync.dma_start(adjT[:, 4:8, :], adjT_src[:, 4:8, :])
            xrTb = P.tile([128, RB], BF16, tag="xrTb")
            nc.gpsimd.dma_start(xrTb[:], d_xrTb[:, :])
            ws1Tb = P.tile([128, H1 * F1], BF16, tag="ws1Tb")
            nc.gpsimd.dma_start(ws1Tb[:], d_ws1Tb[:, :])
            nc.sync.dma_start(adjT[:, 8:12, :], adjT_src[:, 8:12, :])
            b1f = P.tile([128, 4], F32, tag="b1f")
            nc.gpsimd.dma_start(b1f[:], d_b1.rearrange("(k p) -> p k", p=128))
            nc.sync.dma_start(adjT[:, 12:16, :], adjT_src[:, 12:16, :])
            w2n = P.tile([F1, H1 * F1], F32, tag="w2n")
            nc.gpsimd.dma_start(w2n[:], d_w2n[:, :])
            w2Tb = P.tile([128, 4, F1], BF16, tag="w2Tb")
            nc.gpsimd.dma_start(w2Tb[:], d_w2Tb.rearrange("(k p) f -> p k f", p=128))
            ws2Tb = P.tile([128, 4, F1], BF16, tag="ws2Tb")
            nc.gpsimd.dma_start(ws2Tb[:], d_ws2Tb.rearrange("(k p) f -> p k f", p=128))
            a2p = P.tile([F1, 2], F32, tag="a2p")
            nc.gpsimd.dma_start(a2p[:], d_a2p[:, :])
            b2f = P.tile([F1, 1], F32, tag="b2f")
            nc.gpsimd.dma_start(b2f[:], d_b2.ap().rearrange("(f o) -> f o", o=1))

            # constants
            ones1b = P.tile([1, 128], BF16, tag="ones1b")
            nc.vector.memset(ones1b[:], 1.0)
            onesf1 = P.tile([1, F1], F32, tag="onesf1")
            nc.vector.memset(onesf1[:], 1.0)
            ones16 = P.tile([16, 128], BF16, tag="ones16")
            nc.vector.memset(ones16[:], 1.0)
            sel = P.tile([16, H1 * 128], BF16, tag="sel")
            for h in range(H1):
                nc.gpsimd.affine_select(sel[:, h * 128:(h + 1) * 128],
                                        ones16[:], [[0, 128]], ALU.is_equal,
                                        0.0, base=-h, channel_multiplier=1)


            # ============ small exact fp32 matmuls ==========================
            # w1tilde [c=128, 16]: col h = W1_h^T a_src1[h], col 8+h tgt
            ps_wt = PS.tile([128, 512], F32, tag="ps")
            for h in range(H1):
                kt, pr = (h * F1) // 128, (h * F1) % 128
                w1slc = w1n[pr:pr + F1, kt, :]
                nc.tensor.matmul(ps_wt[0:128, h:h + 1], w1slc,
                                 a1sT[pr:pr + F1, h:h + 1])
                nc.tensor.matmul(ps_wt[0:128, 8 + h:9 + h], w1slc,
                                 a1tT[pr:pr + F1, h:h + 1])
            w1t = P.tile([128, 16], F32, tag="w1t")
            nc.vector.tensor_copy(w1t[:], ps_wt[0:128, 0:16])

            # s_src rows for our block -> g (bf16) [16, 512]
            ps_s1r = PS.tile([128, 512], F32, tag="ps")
            nc.tensor.matmul(ps_s1r[0:16, 0:RB], w1t[:], xrT[:])
            g1b = P.tile([16, RB], BF16, tag="g1b")
            nc.scalar.activation(g1b[:], ps_s1r[0:16, 0:RB], AF.Exp, scale=-0.8)

            # S1T [j(128 x 16 chunks), 16] = x @ w1tilde; exp tables per chunk
            ps_s1t = PS.tile([128, 512], F32, tag="ps")
            Wvf = P.tile([128, NJT * 16], F32, tag="Wvf")
            Dvf = P.tile([128, NJT * 16], F32, tag="Dvf")
            for cc in range(4):
                for jc in range(cc * 4, cc * 4 + 4):
                    nc.tensor.matmul(ps_s1t[0:128, jc * 16:(jc + 1) * 16],
                                     xT[:, jc * 128:(jc + 1) * 128], w1t[:])
                cs = slice(cc * 64, (cc + 1) * 64)
                nc.scalar.activation(Wvf[:, cs], ps_s1t[0:128, cs],
                                     AF.Exp, scale=0.8)
                nc.scalar.activation(Dvf[:, cs], ps_s1t[0:128, cs],
                                     AF.Exp, scale=0.2)

            # tables for ACT-side layer-1 q ops: q = relu(g - W)*D + W*D
            W1neg = P.tile([128, NJT * 16], F32, tag="W1neg")
            nc.vector.tensor_scalar(W1neg[:], Wvf[:], -1.0, None, ALU.mult)
            E1v = P.tile([128, NJT * 16], F32, tag="E1v")
            nc.vector.tensor_tensor(E1v[:], Wvf[:], Dvf[:], ALU.mult)

            # ============ layer-1 skip:  (x_R @ Wskip1^T)^T  ================
            skipTb = P.tile([128, 4, RB], BF16, tag="skipTb")
            for pr in range(4):
                ps_sk = PS.tile([128, 512], F32, tag="ps")
                nc.tensor.matmul(ps_sk[0:128, 0:RB],
                                 ws1Tb[:, pr * 128:(pr + 1) * 128], xrTb[:])
                nc.scalar.activation(skipTb[:, pr, :], ps_sk[0:128, 0:RB],
                                     AF.Identity, bias=b1f[:, pr:pr + 1])

            # ============ proj1 (+ ones col) ================================
            p1e = P.tile([128, NJT, 8 * 66], BF16, tag="p1e")
            nc.vector.memset(
                p1e[:].rearrange("p j (h q) -> p j h q", q=66)[:, :, :, 64:65],
                1.0)
            for jt in range(NJT):
                ps_p = PSB.tile([128, 512], F32, tag="psb")
                nc.tensor.matmul(ps_p[0:128, 0:512],
                                 xTb[:, jt * 128:(jt + 1) * 128], w1Tb[:])
                dst = p1e[:, jt, :].rearrange("p (h q) -> p h q", q=66)
                src = ps_p[0:128, 0:512].rearrange("p (h q) -> p h q", q=64)
                if jt % 2 == 0:
                    nc.vector.tensor_copy(dst[:, :, 0:64], src)
                else:
                    nc.scalar.activation(dst[:, :, 0:64], src, AF.Copy)

            # layer-2 w2tilde (weights only; do early)
            ps_w2 = PS.tile([128, 512], F32, tag="ps")
            for kt in range(4):
                nc.tensor.matmul(ps_w2[0:128, kt * 2:kt * 2 + 2],
                                 w2n[:, kt * 128:(kt + 1) * 128], a2p[:],
                                 start=True, stop=True)
            w2tb = P.tile([128, 8], BF16, tag="w2tb")
            nc.vector.tensor_copy(w2tb[:], ps_w2[0:128, 0:8])

            # ============ layer-1 head loop =================================
            numb = P.tile([128, 4, RB], BF16, tag="numb")
            den_pairs = []
            for h in range(H1):
                ps_g = PS.tile([128, 512], F32, tag="ps")
                nc.tensor.matmul(ps_g[0:128, 0:RB],
                                 sel[:, h * 128:(h + 1) * 128], g1b[:])
                gbh = GB.tile([128, RB], BF16, tag="gb")
                nc.scalar.activation(gbh[:], ps_g[0:128, 0:RB], AF.Copy)

                if h % 2 == 0:
                    den_pair = P.tile([1, 2, RB], F32, tag=f"den{h // 2}")
                    den_pairs.append(den_pair)
                numT = PSN.tile([65, 512], F32, tag="numT")
                for grp in range(4):
                    q = WK.tile([128, 4, RB], BF16, tag="q")
                    Ft = WK.tile([128, 4, RB], BF16, tag="F")
                    for k in range(4):
                        jt = grp * 4 + k
                        col = slice(jt * 16 + 8 + h, jt * 16 + 9 + h)
                        if jt in _TSP_ACT1:
                            r1 = WK.tile([128, RB], BF16, tag="r2")
                            nc.scalar.activation(r1[:], gbh[:], AF.Relu,
                                                 bias=W1neg[:, col])
                            nc.scalar.activation(q[:, k, :], r1[:],
                                                 AF.Identity,
                                                 scale=Dvf[:, col],
                                                 bias=E1v[:, col])
                        else:
                            teng = nc.gpsimd if jt in _TSP_POOL1 else nc.vector
                            teng.tensor_scalar(q[:, k, :], gbh[:], Wvf[:, col],
                                               Dvf[:, col], ALU.max, ALU.mult)
                    geng = nc.gpsimd if grp in _TT_POOL1 else nc.vector
                    geng.tensor_tensor(Ft[:], q[:],
                                       adjT[:, grp * 4:grp * 4 + 4, :], ALU.mult)
                    for k in range(4):
                        jt = grp * 4 + k
                        nc.tensor.matmul(numT[0:65, 0:RB],
                                         p1e[:, jt, h * 66:h * 66 + 65],
                                         Ft[:, k, :],
                                         start=(jt == 0), stop=(jt == NJT - 1))
                nc.scalar.activation(den_pairs[h // 2][0:1, h % 2, :],
                                     numT[64:65, 0:RB], AF.Copy)
                nc.scalar.activation(numb[(h % 2) * 64:(h % 2) * 64 + 64, h // 2, :],
                                     numT[0:64, 0:RB], AF.Copy)

            # h_out^T = elu(num/den + (skip + b1)), kept bf16, per pair so
            # late pairs overlap earlier heads' compute
            houtb = P.tile([128, 4, RB], BF16, tag="houtb")
            for pr in range(4):
                rec_p = P.tile([1, 2, RB], F32, tag=f"rec{pr}")
                nc.vector.reciprocal_approx_fast(rec_p[:], den_pairs[pr][:])
                rdb = GB.tile([128, RB], BF16, tag="gb")
                ps_r = PS.tile([128, 512], F32, tag="ps")
                nc.tensor.matmul(ps_r[0:64, 0:RB], onesf1[:], rec_p[0:1, 0, :])
                nc.tensor.matmul(ps_r[64:128, 0:RB], onesf1[:], rec_p[0:1, 1, :])
                nc.scalar.activation(rdb[:], ps_r[0:128, 0:RB], AF.Copy)
                hpre = WK.tile([128, RB], BF16, tag="hpre")
                heng = nc.gpsimd if pr % 2 == 0 else nc.vector
                heng.tensor_mul(hpre[:], numb[:, pr, :], rdb[:])
                u = WK.tile([128, RB], BF16, tag="u")
                heng.tensor_add(u[:], hpre[:], skipTb[:, pr, :])
                m0 = WK.tile([128, RB], BF16, tag="hpre")
                nc.vector.tensor_scalar(m0[:], u[:], 0.0, None, ALU.min)
                e = WK.tile([128, RB], BF16, tag="e")
                nc.scalar.activation(e[:], m0[:], AF.Exp)
                nc.vector.scalar_tensor_tensor(
                    houtb[:, pr, :], e[:], -1.0, u[:], ALU.add, ALU.max)

            # ============ layer-2 local pieces ==============================
            # S2: s_src2 -> psum row 0, s_tgt2 -> psum row 32
            ps_s2 = PS.tile([128, 512], F32, tag="ps")
            for kt in range(4):
                nc.tensor.matmul(ps_s2[0:1, 0:RB], w2tb[:, kt * 2:kt * 2 + 1],
                                 houtb[:, kt, :], start=(kt == 0), stop=(kt == 3))
            for kt in range(4):
                nc.tensor.matmul(ps_s2[32:33, 0:RB], w2tb[:, kt * 2 + 1:kt * 2 + 2],
                                 houtb[:, kt, :], start=(kt == 0), stop=(kt == 3))
            g2row = P.tile([1, RB], BF16, tag="g2row")
            nc.scalar.activation(g2row[:], ps_s2[0:1, 0:RB], AF.Exp, scale=-0.8)
            stg2b = P.tile([1, RB], BF16, tag="stg2b")
            nc.scalar.activation(stg2b[:], ps_s2[32:33, 0:RB], AF.Copy)

            # proj2^T local [64, 512] in bf16 for the gather
            ps_p2 = PS.tile([128, 512], F32, tag="ps")
            for kt in range(4):
                nc.tensor.matmul(ps_p2[0:64, 0:RB], w2Tb[:, kt, :],
                                 houtb[:, kt, :], start=(kt == 0), stop=(kt == 3))
            p2Tb = P.tile([F1, RB], BF16, tag="p2Tb")
            nc.scalar.activation(p2Tb[:], ps_p2[0:64, 0:RB], AF.Copy)

            # ---- gather-independent layer-2 prep, before the collective ----
            p2e = P.tile([128, NJT, F1 + 1], BF16, tag="p2e")
            ps_g2 = PS.tile([128, 512], F32, tag="ps")
            nc.tensor.matmul(ps_g2[0:128, 0:RB], ones1b[:], g2row[:])
            g2bc = GB.tile([128, RB], BF16, tag="gb")
            nc.scalar.activation(g2bc[:], ps_g2[0:128, 0:RB], AF.Copy)
            ps_sk2 = PS.tile([128, 512], F32, tag="ps")
            for kt in range(4):
                nc.tensor.matmul(ps_sk2[0:64, 0:RB], ws2Tb[:, kt, :],
                                 houtb[:, kt, :], start=(kt == 0), stop=(kt == 3))

            # ============ AllGather within batch group ======================
            # gin bf16 [4*65, 128]: rows (s, 0..63) = proj2^T slices,
            # row (s, 64) = s_tgt2
            gin = DR.tile([4 * (F1 + 1), 128], BF16)
            gin_v = gin.rearrange("(s f) p -> s f p", f=F1 + 1)
            nc.sync.dma_start(
                gin_v[:, 0:F1, :].rearrange("s f p -> f s p"),
                p2Tb[:].rearrange("f (s p) -> f s p", p=128))
            nc.scalar.dma_start(
                gin_v[:, F1:F1 + 1, :].rearrange("s o p -> o s p"),
                stg2b[:].rearrange("o (s p) -> o s p", p=128))
            gout = DR.tile([4 * 4 * (F1 + 1), 128], BF16)
            nc.gpsimd.collective_compute(
                "AllGather", ALU.bypass, replica_groups=GROUPS,
                ins=[gin.opt()], outs=[gout.opt()])
            gout_v = gout.rearrange("(c s f) p -> c s f p", s=4, f=F1 + 1)

            # ============ layer-2 attention =================================
            nc.sync.dma_start(
                p2e[:],
                gout_v.rearrange("c s f p -> p (c s) f"))
            st2Tb = P.tile([128, 4, 4], BF16, tag="st2Tb")
            nc.scalar.dma_start(
                st2Tb[:], gout_v[:, :, F1, :].rearrange("c s p -> p c s"))
            nc.vector.memset(p2e[:, :, F1:F1 + 1], 1.0)  # denominator column
            W2vf = P.tile([128, 16], F32, tag="W2vf")
            nc.scalar.activation(W2vf[:], st2Tb[:].rearrange("p c s -> p (c s)"),
                                 AF.Exp, scale=0.8)
            D2v = P.tile([128, 16], F32, tag="D2v")
            nc.scalar.activation(D2v[:], st2Tb[:].rearrange("p c s -> p (c s)"),
                                 AF.Exp, scale=0.2)
            # tables for the ACT-side q ops: q = (relu(g - W) * D + W*D)
            W2neg = P.tile([128, 16], F32, tag="W2neg")
            nc.vector.tensor_scalar(W2neg[:], W2vf[:], -1.0, None, ALU.mult)
            E2v = P.tile([128, 16], F32, tag="E2v")
            nc.vector.tensor_tensor(E2v[:], W2vf[:], D2v[:], ALU.mult)

            numT2 = PSN.tile([65, 512], F32, tag="numT")
            for grp in range(4):
                q2 = WK.tile([128, 4, RB], BF16, tag="q")
                F2t = WK.tile([128, 4, RB], BF16, tag="F")
                for k in range(4):
                    jt = grp * 4 + k
                    col = slice(jt, jt + 1)
                    if jt in _TSP_ACT2:
                        r2 = WK.tile([128, RB], BF16, tag="r2")
                        nc.scalar.activation(r2[:], g2bc[:], AF.Relu,
                                             bias=W2neg[:, col])
                        nc.scalar.activation(q2[:, k, :], r2[:], AF.Identity,
                                             scale=D2v[:, col],
                                             bias=E2v[:, col])
                    else:
                        teng = nc.gpsimd if jt in _TSP_POOL2 else nc.vector
                        teng.tensor_scalar(q2[:, k, :], g2bc[:], W2vf[:, col],
                                           D2v[:, col], ALU.max, ALU.mult)
                geng = nc.gpsimd if grp in _TT_POOL2 else nc.vector
                geng.tensor_tensor(F2t[:], q2[:],
                                   adjT[:, grp * 4:grp * 4 + 4, :], ALU.mult)
                for k in range(4):
                    jt = grp * 4 + k
                    nc.tensor.matmul(numT2[0:F1 + 1, 0:RB],
                                     p2e[:, jt, 0:F1 + 1], F2t[:, k, :],
                                     start=(jt == 0), stop=(jt == NJT - 1))

            den2 = P.tile([1, RB], F32, tag="den2")
            nc.scalar.activation(den2[:], numT2[F1:F1 + 1, 0:RB], AF.Copy)
            rec2 = P.tile([1, RB], F32, tag="rec2")
            nc.vector.reciprocal_approx_fast(rec2[:], den2[:])
            ps_r2 = PS.tile([128, 512], F32, tag="ps")
            nc.tensor.matmul(ps_r2[0:64, 0:RB], onesf1[:], rec2[:])
            rdb2 = GB.tile([128, RB], BF16, tag="rdb")
            nc.vector.tensor_copy(rdb2[0:64, :], ps_r2[0:64, 0:RB])

            t2 = WK.tile([F1, RB], F32, tag="t2")
            nc.vector.tensor_mul(t2[:], numT2[0:F1, 0:RB], rdb2[0:64, :])
            o2 = WK.tile([F1, RB], F32, tag="o2")
            nc.vector.scalar_tensor_tensor(
                o2[:], t2[:], b2f[:], ps_sk2[0:64, 0:RB], ALU.add, ALU.add)
            nc.sync.dma_start(d_out[:, :], o2[:])

    nc.compile()
    return nc


_NC_CACHE = None


def _get_nc():
    global _NC_CACHE
    if _NC_CACHE is None:
        _NC_CACHE = build_nc()
    return _NC_CACHE


def make_in_maps(x, adj, W1, a_src1, a_tgt1, Wskip1, b1, W2, a_src2, a_tgt2,
                 Wskip2, b2):
    x = np.asarray(x, np.float32)
    adj = np.asarray(adj, np.float32)
    W1 = np.asarray(W1, np.float32)
    W2 = np.asarray(W2, np.float32)
    Wskip1 = np.asarray(Wskip1, np.float32)
    Wskip2 = np.asarray(Wskip2, np.float32)
    w1Tb = np.ascontiguousarray(W1.T).astype(BF16NP)
    ws1Tb = np.ascontiguousarray(Wskip1.T).astype(BF16NP)
    w2Tb = np.ascontiguousarray(W2.T).astype(BF16NP)
    ws2Tb = np.ascontiguousarray(Wskip2.T).astype(BF16NP)
    a1s2 = np.ascontiguousarray(
        np.vstack([np.asarray(a_src1, np.float32).T] * 2))
    a1t2 = np.ascontiguousarray(
        np.vstack([np.asarray(a_tgt1, np.float32).T] * 2))
    a2p = np.ascontiguousarray(
        np.stack([np.asarray(a_src2, np.float32).ravel(),
                  np.asarray(a_tgt2, np.float32).ravel()], axis=1))
    xT = [np.ascontiguousarray(x[b].T) for b in range(BS)]
    xTb = [t.astype(BF16NP) for t in xT]
    adjTb = [np.ascontiguousarray(adj[b].T.astype(BF16NP)) for b in range(BS)]
    in_maps = []
    for c in range(NCORES):
        b, r = c // 4, c % 4
        sl = slice(r * RB, (r + 1) * RB)
        xrT = np.ascontiguousarray(xT[b][:, sl])
        in_maps.append({
            "adjT": np.ascontiguousarray(adjTb[b][:, sl]),
            "xT": xT[b], "xTb": xTb[b],
            "xrT": xrT, "xrTb": xrT.astype(BF16NP),
            "w1n": W1, "w1Tb": w1Tb, "ws1Tb": ws1Tb,
            "a1s2": a1s2, "a1t2": a1t2,
            "b1": np.asarray(b1, np.float32),
            "w2n": W2, "w2Tb": w2Tb, "ws2Tb": ws2Tb,
            "a2p": a2p,
            "b2": np.asarray(b2, np.float32),
        })
    return in_maps


def kernel(x, adj, W1, a_src1, a_tgt1, Wskip1, b1, W2, a_src2, a_tgt2,
           Wskip2, b2):
    nc = _get_nc()
    in_maps = make_in_maps(x, adj, W1, a_src1, a_tgt1, Wskip1, b1, W2,
                           a_src2, a_tgt2, Wskip2, b2)
    res = run_bass_kernel_spmd(nc, in_maps, core_ids=list(range(NCORES)))
    out = np.empty((BS, N, F1), np.float32)
    for c in range(NCORES):
        b, r = c // 4, c % 4
        out[b, r * RB:(r + 1) * RB, :] = res.results[c]["outT"].T
    return out


# revision 37
# speedup vs baseline: 1.0070x; 1.0070x over previous
"""GAT (2-layer graph attention) Trainium2 Bass kernel, 8-core SPMD.

Sharding: data-parallel over batch (2) x row-blocks (4) -> 8 cores.
Core c handles batch b=c//4, output rows R=[512*(c%4), 512*(c%4+1)).

Key algebra: with z = s_src[i]+s_tgt[j], the GAT edge weight
exp(leaky_relu(z, 0.2)) = max(exp(z), exp(0.2 z)).  Softmax rows are
invariant to a per-row scale, so dividing row i by exp(s_src[i]) gives
unnormalized weights F[j,i] = adj[i,j] * D[j] * max(W[j], g[i]) with
  W[j] = exp(0.8 s_tgt[j]),  D[j] = exp(0.2 s_tgt[j]),  g[i] = exp(-0.8 s_src[i])
-- no per-element transcendentals.  A ones-column in the stationary
operand makes the softmax denominator fall out as a matmul row.

Host prep does layout only (transpose / cast / slice, no math): adj
arrives pre-transposed per core as bf16, x transposed in f32 (scores)
and bf16 (projection), weights pre-transposed in bf16.  Inner work per
(head, j-tile) is one tensor_scalar (4x DVE mode) + one 4-tile-merged
tensor_tensor feeding the TensorE numerator matmul in bf16.

Layer-1 outputs are exchanged within each batch group of 4 cores via a
single AllGather of (proj2^T | s_tgt2).  Work that does not need the
gather is emitted before the collective (tile serializes around it).
"""

import os
import sys

for _p in ("/opt/trn_rl_repo", "/root/.axon_site/_ro/trn_rl_repo"):
    if os.path.isdir(_p) and _p not in sys.path:
        sys.path.insert(0, _p)

import numpy as np
import ml_dtypes

import concourse.bass as bass
import concourse.bacc as bacc
import concourse.mybir as mybir
from concourse import tile
from concourse.bass_utils import run_bass_kernel_spmd

F32 = mybir.dt.float32
BF16 = mybir.dt.bfloat16
AF = mybir.ActivationFunctionType
ALU = mybir.AluOpType
BF16NP = ml_dtypes.bfloat16

BS, N, FIN = 2, 2048, 128
H1, F1 = 8, 64
RB = 512          # row block per core
NJT = N // 128    # 16 j-tiles
NCORES = 8
GROUPS = [[0, 1, 2, 3], [4, 5, 6, 7]]

# layer-1 inner-loop engine split (per head): 16 TSPs, 4 merged TTs
_TSP_POOL1 = {1, 3, 6, 9, 11, 14, 15}   # 7 TSPs per head on Pool
_TT_POOL1 = {3}                          # TT group 3 on Pool, 0-2 DVE
# layer-2: 16 TSPs (4 on ACT via relu/affine pair), 4 merged TTs
_TSP_POOL2 = {1, 3, 5, 9, 13}
_TSP_ACT2 = {7, 15}
_TT_POOL2 = {1, 3}


def build_nc():
    nc = bacc.Bacc("TRN2", target_bir_lowering=False, debug=False,
                   num_devices=NCORES)

    # ---- per-core DRAM I/O (host gives transposed / bf16 layouts) ----
    d_adjT = nc.declare_dram_parameter("adjT", [N, RB], BF16, isOutput=False)
    d_xT = nc.declare_dram_parameter("xT", [FIN, N], F32, isOutput=False)
    d_xTb = nc.declare_dram_parameter("xTb", [FIN, N], BF16, isOutput=False)
    d_xrT = nc.declare_dram_parameter("xrT", [FIN, RB], F32, isOutput=False)
    d_xrTb = nc.declare_dram_parameter("xrTb", [FIN, RB], BF16, isOutput=False)
    d_w1n = nc.declare_dram_parameter("w1n", [H1 * F1, FIN], F32, isOutput=False)
    d_w1Tb = nc.declare_dram_parameter("w1Tb", [FIN, H1 * F1], BF16,
                                       isOutput=False)
    d_ws1Tb = nc.declare_dram_parameter("ws1Tb", [FIN, H1 * F1], BF16,
                                        isOutput=False)
    d_a1s2 = nc.declare_dram_parameter("a1s2", [128, H1], F32, isOutput=False)
    d_a1t2 = nc.declare_dram_parameter("a1t2", [128, H1], F32, isOutput=False)
    d_b1 = nc.declare_dram_parameter("b1", [H1 * F1], F32, isOutput=False)
    d_w2n = nc.declare_dram_parameter("w2n", [F1, H1 * F1], F32, isOutput=False)
    d_w2Tb = nc.declare_dram_parameter("w2Tb", [H1 * F1, F1], BF16,
                                       isOutput=False)
    d_ws2Tb = nc.declare_dram_parameter("ws2Tb", [H1 * F1, F1], BF16,
                                        isOutput=False)
    d_a2p = nc.declare_dram_parameter("a2p", [F1, 2], F32, isOutput=False)
    d_b2 = nc.declare_dram_parameter("b2", [F1], F32, isOutput=False)
    # output: transposed row-block out^T [64, 512] (host transposes back)
    d_out = nc.declare_dram_parameter("outT", [F1, RB], F32, isOutput=True)

    with tile.TileContext(nc) as tc:
        with (
            tc.tile_pool(name="persist", bufs=1) as P,
            tc.tile_pool(name="work", bufs=4) as WK,
            tc.tile_pool(name="gbp", bufs=3) as GB,
            tc.tile_pool(name="ps", bufs=3, space="PSUM") as PS,
            tc.tile_pool(name="psb", bufs=2, space="PSUM") as PSB,
            tc.tile_pool(name="psnum", bufs=3, space="PSUM") as PSN,
            tc.tile_pool(name="dram", bufs=1, space="DRAM") as DR,
        ):
            # ============ loads (emission order ~= DMA priority) ============
            # big/early transfers all on the otherwise-idle SP queue so the
            # ACT/Pool sequencers stay free for compute
            w1n = P.tile([128, 4, FIN], F32, tag="w1n")
            nc.sync.dma_start(w1n[:], d_w1n.rearrange("(k p) c -> p k c", p=128))
            a1sT = P.tile([128, H1], F32, tag="a1sT")
            nc.scalar.dma_start(a1sT[:], d_a1s2[:, :])
            a1tT = P.tile([128, H1], F32, tag="a1tT")
            nc.gpsimd.dma_start(a1tT[:], d_a1t2[:, :])
            xrT = P.tile([128, RB], F32, tag="xrT")
            nc.sync.dma_start(xrT[:], d_xrT[:, :])

            xT = P.tile([128, N], F32, tag="xT")
            nc.sync.dma_start(xT[:, 0:1024], d_xT[:, 0:1024])
            w1Tb = P.tile([128, H1 * F1], BF16, tag="w1Tb")
            nc.gpsimd.dma_start(w1Tb[:], d_w1Tb[:, :])
            xTb = P.tile([128, N], BF16, tag="xTb")
            nc.scalar.dma_start(xTb[:, 0:1024], d_xTb[:, 0:1024])
            adjT = P.tile([128, NJT, RB], BF16, tag="adjT")
            adjT_src = d_adjT.rearrange("(t p) i -> p t i", p=128)
            nc.sync.dma_start(adjT[:, 0:4, :], adjT_src[:, 0:4, :])
            nc.sync.dma_start(xT[:, 1024:2048], d_xT[:, 1024:2048])
            nc.scalar.dma_start(xTb[:, 1024:2048], d_xTb[:, 1024:2048])
            nc.sync.dma_start(adjT[:, 4:8, :], adjT_src[:, 4:8, :])
            xrTb = P.tile([128, RB], BF16, tag="xrTb")
            nc.gpsimd.dma_start(xrTb[:], d_xrTb[:, :])
            ws1Tb = P.tile([128, H1 * F1], BF16, tag="ws1Tb")
            nc.gpsimd.dma_start(ws1Tb[:], d_ws1Tb[:, :])
            nc.sync.dma_start(adjT[:, 8:12, :], adjT_src[:, 8:12, :])
            b1f = P.tile([128, 4], F32, tag="b1f")
            nc.gpsimd.dma_start(b1f[:], d_b1.rearrange("(k p) -> p k", p=128))
            nc.sync.dma_start(adjT[:, 12:16, :], adjT_src[:, 12:16, :])
            w2n = P.tile([F1, H1 * F1], F32, tag="w2n")
            nc.gpsimd.dma_start(w2n[:], d_w2n[:, :])
            w2Tb = P.tile([128, 4, F1], BF16, tag="w2Tb")
            nc.gpsimd.dma_start(w2Tb[:], d_w2Tb.rearrange("(k p) f -> p k f", p=128))
            ws2Tb = P.tile([128, 4, F1], BF16, tag="ws2Tb")
            nc.gpsimd.dma_start(ws2Tb[:], d_ws2Tb.rearrange("(k p) f -> p k f", p=128))
            a2p = P.tile([F1, 2], F32, tag="a2p")
            nc.gpsimd.dma_start(a2p[:], d_a2p[:, :])
            b2f = P.tile([F1, 1], F32, tag="b2f")
            nc.gpsimd.dma_start(b2f[:], d_b2.ap().rearrange("(f o) -> f o", o=1))

            # constants
            ones1b = P.tile([1, 128], BF16, tag="ones1b")
            nc.vector.memset(ones1b[:], 1.0)
            onesf1 = P.tile([1, F1], F32, tag="onesf1")
            nc.vector.memset(onesf1[:], 1.0)
            ones16 = P.tile([16, 128], BF16, tag="ones16")
            nc.vector.memset(ones16[:], 1.0)
            sel = P.tile([16, H1 * 128], BF16, tag="sel")
            for h in range(H1):
                nc.gpsimd.affine_select(sel[:, h * 128:(h + 1) * 128],
                                        ones16[:], [[0, 128]], ALU.is_equal,
                                        0.0, base=-h, channel_multiplier=1)


            # ============ small exact fp32 matmuls ==========================
            # w1tilde [c=128, 16]: col h = W1_h^T a_src1[h], col 8+h tgt
            ps_wt = PS.tile([128, 512], F32, tag="ps")
            for h in range(H1):
                kt, pr = (h * F1) // 128, (h * F1) % 128
                w1slc = w1n[pr:pr + F1, kt, :]
                nc.tensor.matmul(ps_wt[0:128, h:h + 1], w1slc,
                                 a1sT[pr:pr + F1, h:h + 1])
                nc.tensor.matmul(ps_wt[0:128, 8 + h:9 + h], w1slc,
                                 a1tT[pr:pr + F1, h:h + 1])
            w1t = P.tile([128, 16], F32, tag="w1t")
            nc.vector.tensor_copy(w1t[:], ps_wt[0:128, 0:16])

            # s_src rows for our block -> g (bf16) [16, 512]
            ps_s1r = PS.tile([128, 512], F32, tag="ps")
            nc.tensor.matmul(ps_s1r[0:16, 0:RB], w1t[:], xrT[:])
            g1b = P.tile([16, RB], BF16, tag="g1b")
            nc.scalar.activation(g1b[:], ps_s1r[0:16, 0:RB], AF.Exp, scale=-0.8)

            # S1T [j(128 x 16 chunks), 16] = x @ w1tilde; exp tables per chunk
            ps_s1t = PS.tile([128, 512], F32, tag="ps")
            Wvf = P.tile([128, NJT * 16], F32, tag="Wvf")
            Dvf = P.tile([128, NJT * 16], F32, tag="Dvf")
            for cc in range(4):
                for jc in range(cc * 4, cc * 4 + 4):
                    nc.tensor.matmul(ps_s1t[0:128, jc * 16:(jc + 1) * 16],
                                     xT[:, jc * 128:(jc + 1) * 128], w1t[:])
                cs = slice(cc * 64, (cc + 1) * 64)
                nc.scalar.activation(Wvf[:, cs], ps_s1t[0:128, cs],
                                     AF.Exp, scale=0.8)
                nc.scalar.activation(Dvf[:, cs], ps_s1t[0:128, cs],
                                     AF.Exp, scale=0.2)

            # ============ layer-1 skip:  (x_R @ Wskip1^T)^T  ================
            skipTb = P.tile([128, 4, RB], BF16, tag="skipTb")
            for pr in range(4):
                ps_sk = PS.tile([128, 512], F32, tag="ps")
                nc.tensor.matmul(ps_sk[0:128, 0:RB],
                                 ws1Tb[:, pr * 128:(pr + 1) * 128], xrTb[:])
                nc.scalar.activation(skipTb[:, pr, :], ps_sk[0:128, 0:RB],
                                     AF.Identity, bias=b1f[:, pr:pr + 1])

            # ============ proj1 (+ ones col) ================================
            p1e = P.tile([128, NJT, 8 * 66], BF16, tag="p1e")
            nc.vector.memset(
                p1e[:].rearrange("p j (h q) -> p j h q", q=66)[:, :, :, 64:65],
                1.0)
            for jt in range(NJT):
                ps_p = PSB.tile([128, 512], F32, tag="psb")
                nc.tensor.matmul(ps_p[0:128, 0:512],
                                 xTb[:, jt * 128:(jt + 1) * 128], w1Tb[:])
                dst = p1e[:, jt, :].rearrange("p (h q) -> p h q", q=66)
                src = ps_p[0:128, 0:512].rearrange("p (h q) -> p h q", q=64)
                if jt % 2 == 0:
                    nc.vector.tensor_copy(dst[:, :, 0:64], src)
                else:
                    nc.scalar.activation(dst[:, :, 0:64], src, AF.Copy)

            # layer-2 w2tilde (weights only; do early)
            ps_w2 = PS.tile([128, 512], F32, tag="ps")
            for kt in range(4):
                nc.tensor.matmul(ps_w2[0:128, kt * 2:kt * 2 + 2],
                                 w2n[:, kt * 128:(kt + 1) * 128], a2p[:],
                                 start=True, stop=True)
            w2tb = P.tile([128, 8], BF16, tag="w2tb")
            nc.vector.tensor_copy(w2tb[:], ps_w2[0:128, 0:8])

            # ============ layer-1 head loop =================================
            numb = P.tile([128, 4, RB], BF16, tag="numb")
            den_pairs = []
            for h in range(H1):
                ps_g = PS.tile([128, 512], F32, tag="ps")
                nc.tensor.matmul(ps_g[0:128, 0:RB],
                                 sel[:, h * 128:(h + 1) * 128], g1b[:])
                gbh = GB.tile([128, RB], BF16, tag="gb")
                nc.scalar.activation(gbh[:], ps_g[0:128, 0:RB], AF.Copy)

                if h % 2 == 0:
                    den_pair = P.tile([1, 2, RB], F32, tag=f"den{h // 2}")
                    den_pairs.append(den_pair)
                numT = PSN.tile([65, 512], F32, tag="numT")
                for grp in range(4):
                    q = WK.tile([128, 4, RB], BF16, tag="q")
                    Ft = WK.tile([128, 4, RB], BF16, tag="F")
                    for k in range(4):
                        jt = grp * 4 + k
                        col = slice(jt * 16 + 8 + h, jt * 16 + 9 + h)
                        teng = nc.gpsimd if jt in _TSP_POOL1 else nc.vector
                        teng.tensor_scalar(q[:, k, :], gbh[:], Wvf[:, col],
                                           Dvf[:, col], ALU.max, ALU.mult)
                    geng = nc.gpsimd if grp in _TT_POOL1 else nc.vector
                    geng.tensor_tensor(Ft[:], q[:],
                                       adjT[:, grp * 4:grp * 4 + 4, :], ALU.mult)
                    for k in range(4):
                        jt = grp * 4 + k
                        nc.tensor.matmul(numT[0:65, 0:RB],
                                         p1e[:, jt, h * 66:h * 66 + 65],
                                         Ft[:, k, :],
                                         start=(jt == 0), stop=(jt == NJT - 1))
                nc.scalar.activation(den_pairs[h // 2][0:1, h % 2, :],
                                     numT[64:65, 0:RB], AF.Copy)
                nc.scalar.activation(numb[(h % 2) * 64:(h % 2) * 64 + 64, h // 2, :],
                                     numT[0:64, 0:RB], AF.Copy)

            # h_out^T = elu(num/den + (skip + b1)), kept bf16, per pair so
            # late pairs overlap earlier heads' compute
            houtb = P.tile([128, 4, RB], BF16, tag="houtb")
            for pr in range(4):
                rec_p = P.tile([1, 2, RB], F32, tag=f"rec{pr}")
                nc.vector.reciprocal_approx_fast(rec_p[:], den_pairs[pr][:])
                rdb = GB.tile([128, RB], BF16, tag="gb")
                ps_r = PS.tile([128, 512], F32, tag="ps")
                nc.tensor.matmul(ps_r[0:64, 0:RB], onesf1[:], rec_p[0:1, 0, :])
                nc.tensor.matmul(ps_r[64:128, 0:RB], onesf1[:], rec_p[0:1, 1, :])
                nc.scalar.activation(rdb[:], ps_r[0:128, 0:RB], AF.Copy)
                hpre = WK.tile([128, RB], BF16, tag="hpre")
                heng = nc.gpsimd if pr % 2 == 0 else nc.vector
                heng.tensor_mul(hpre[:], numb[:, pr, :], rdb[:])
                u = WK.tile([128, RB], BF16, tag="u")
                heng.tensor_add(u[:], hpre[:], skipTb[:, pr, :])
                m0 = WK.tile([128, RB], BF16, tag="hpre")
                nc.vector.tensor_scalar(m0[:], u[:], 0.0, None, ALU.min)
                e = WK.tile([128, RB], BF16, tag="e")
                nc.scalar.activation(e[:], m0[:], AF.Exp)
                nc.vector.scalar_tensor_tensor(
                    houtb[:, pr, :], e[:], -1.0, u[:], ALU.add, ALU.max)

            # ============ layer-2 local pieces ==============================
            # S2: s_src2 -> psum row 0, s_tgt2 -> psum row 32
            ps_s2 = PS.tile([128, 512], F32, tag="ps")
            for kt in range(4):
                nc.tensor.matmul(ps_s2[0:1, 0:RB], w2tb[:, kt * 2:kt * 2 + 1],
                                 houtb[:, kt, :], start=(kt == 0), stop=(kt == 3))
            for kt in range(4):
                nc.tensor.matmul(ps_s2[32:33, 0:RB], w2tb[:, kt * 2 + 1:kt * 2 + 2],
                                 houtb[:, kt, :], start=(kt == 0), stop=(kt == 3))
            g2row = P.tile([1, RB], BF16, tag="g2row")
            nc.scalar.activation(g2row[:], ps_s2[0:1, 0:RB], AF.Exp, scale=-0.8)
            stg2b = P.tile([1, RB], BF16, tag="stg2b")
            nc.scalar.activation(stg2b[:], ps_s2[32:33, 0:RB], AF.Copy)

            # proj2^T local [64, 512] in bf16 for the gather
            ps_p2 = PS.tile([128, 512], F32, tag="ps")
            for kt in range(4):
                nc.tensor.matmul(ps_p2[0:64, 0:RB], w2Tb[:, kt, :],
                                 houtb[:, kt, :], start=(kt == 0), stop=(kt == 3))
            p2Tb = P.tile([F1, RB], BF16, tag="p2Tb")
            nc.scalar.activation(p2Tb[:], ps_p2[0:64, 0:RB], AF.Copy)

            # ---- gather-independent layer-2 prep, before the collective ----
            p2e = P.tile([128, NJT, F1 + 1], BF16, tag="p2e")
            ps_g2 = PS.tile([128, 512], F32, tag="ps")
            nc.tensor.matmul(ps_g2[0:128, 0:RB], ones1b[:], g2row[:])
            g2bc = GB.tile([128, RB], BF16, tag="gb")
            nc.scalar.activation(g2bc[:], ps_g2[0:128, 0:RB], AF.Copy)
            ps_sk2 = PS.tile([128, 512], F32, tag="ps")
            for kt in range(4):
                nc.tensor.matmul(ps_sk2[0:64, 0:RB], ws2Tb[:, kt, :],
                                 houtb[:, kt, :], start=(kt == 0), stop=(kt == 3))

            # ============ AllGather within batch group ======================
            # gin bf16 [4*65, 128]: rows (s, 0..63) = proj2^T slices,
            # row (s, 64) = s_tgt2
            gin = DR.tile([4 * (F1 + 1), 128], BF16)
            gin_v = gin.rearrange("(s f) p -> s f p", f=F1 + 1)
            nc.sync.dma_start(
                gin_v[:, 0:F1, :].rearrange("s f p -> f s p"),
                p2Tb[:].rearrange("f (s p) -> f s p", p=128))
            nc.scalar.dma_start(
                gin_v[:, F1:F1 + 1, :].rearrange("s o p -> o s p"),
                stg2b[:].rearrange("o (s p) -> o s p", p=128))
            gout = DR.tile([4 * 4 * (F1 + 1), 128], BF16)
            nc.gpsimd.collective_compute(
                "AllGather", ALU.bypass, replica_groups=GROUPS,
                ins=[gin.opt()], outs=[gout.opt()])
            gout_v = gout.rearrange("(c s f) p -> c s f p", s=4, f=F1 + 1)

            # ============ layer-2 attention =================================
            nc.sync.dma_start(
                p2e[:],
                gout_v.rearrange("c s f p -> p (c s) f"))
            st2Tb = P.tile([128, 4, 4], BF16, tag="st2Tb")
            nc.scalar.dma_start(
                st2Tb[:], gout_v[:, :, F1, :].rearrange("c s p -> p c s"))
            nc.vector.memset(p2e[:, :, F1:F1 + 1], 1.0)  # denominator column
            W2vf = P.tile([128, 16], F32, tag="W2vf")
            nc.scalar.activation(W2vf[:], st2Tb[:].rearrange("p c s -> p (c s)"),
                                 AF.Exp, scale=0.8)
            D2v = P.tile([128, 16], F32, tag="D2v")
            nc.scalar.activation(D2v[:], st2Tb[:].rearrange("p c s -> p (c s)"),
                                 AF.Exp, scale=0.2)
            # tables for the ACT-side q ops: q = (relu(g - W) * D + W*D)
            W2neg = P.tile([128, 16], F32, tag="W2neg")
            nc.vector.tensor_scalar(W2neg[:], W2vf[:], -1.0, None, ALU.mult)
            E2v = P.tile([128, 16], F32, tag="E2v")
            nc.vector.tensor_tensor(E2v[:], W2vf[:], D2v[:], ALU.mult)

            numT2 = PSN.tile([65, 512], F32, tag="numT")
            for grp in range(4):
                q2 = WK.tile([128, 4, RB], BF16, tag="q")
                F2t = WK.tile([128, 4, RB], BF16, tag="F")
                for k in range(4):
                    jt = grp * 4 + k
                    col = slice(jt, jt + 1)
                    if jt in _TSP_ACT2:
                        r2 = WK.tile([128, RB], BF16, tag="r2")
                        nc.scalar.activation(r2[:], g2bc[:], AF.Relu,
                                             bias=W2neg[:, col])
                        nc.scalar.activation(q2[:, k, :], r2[:], AF.Identity,
                                             scale=D2v[:, col],
                                             bias=E2v[:, col])
                    else:
                        teng = nc.gpsimd if jt in _TSP_POOL2 else nc.vector
                        teng.tensor_scalar(q2[:, k, :], g2bc[:], W2vf[:, col],
                                           D2v[:, col], ALU.max, ALU.mult)
                geng = nc.gpsimd if grp in _TT_POOL2 else nc.vector
                geng.tensor_tensor(F2t[:], q2[:],
                                   adjT[:, grp * 4:grp * 4 + 4, :], ALU.mult)
                for k in range(4):
                    jt = grp * 4 + k
                    nc.tensor.matmul(numT2[0:F1 + 1, 0:RB],
                                     p2e[:, jt, 0:F1 + 1], F2t[:, k, :],
                                     start=(jt == 0), stop=(jt == NJT - 1))

            den2 = P.tile([1, RB], F32, tag="den2")
            nc.scalar.activation(den2[:], numT2[F1:F1 + 1, 0:RB], AF.Copy)
            rec2 = P.tile([1, RB], F32, tag="rec2")
            nc.vector.reciprocal_approx_fast(rec2[:], den2[:])
            ps_r2 = PS.tile([128, 512], F32, tag="ps")
            nc.tensor.matmul(ps_r2[0:64, 0:RB], onesf1[:], rec2[:])
            rdb2 = GB.tile([128, RB], BF16, tag="rdb")
            nc.vector.tensor_copy(rdb2[0:64, :], ps_r2[0:64, 0:RB])

            t2 = WK.tile([F1, RB], F32, tag="t2")
            nc.vector.tensor_mul(t2[:], numT2[0:F1, 0:RB], rdb2[0:64, :])
            o2 = WK.tile([F1, RB], F32, tag="o2")
            nc.vector.scalar_tensor_tensor(
                o2[:], t2[:], b2f[:], ps_sk2[0:64, 0:RB], ALU.add, ALU.add)
            nc.sync.dma_start(d_out[:, :], o2[:])

    nc.compile()
    return nc


_NC_CACHE = None


def _get_nc():
    global _NC_CACHE
    if _NC_CACHE is None:
        _NC_CACHE = build_nc()
    return _NC_CACHE


def make_in_maps(x, adj, W1, a_src1, a_tgt1, Wskip1, b1, W2, a_src2, a_tgt2,
                 Wskip2, b2):
    x = np.asarray(x, np.float32)
    adj = np.asarray(adj, np.float32)
    W1 = np.asarray(W1, np.float32)
    W2 = np.asarray(W2, np.float32)
    Wskip1 = np.asarray(Wskip1, np.float32)
    Wskip2 = np.asarray(Wskip2, np.float32)
    w1Tb = np.ascontiguousarray(W1.T).astype(BF16NP)
    ws1Tb = np.ascontiguousarray(Wskip1.T).astype(BF16NP)
    w2Tb = np.ascontiguousarray(W2.T).astype(BF16NP)
    ws2Tb = np.ascontiguousarray(Wskip2.T).astype(BF16NP)
    a1s2 = np.ascontiguousarray(
        np.vstack([np.asarray(a_src1, np.float32).T] * 2))
    a1t2 = np.ascontiguousarray(
        np.vstack([np.asarray(a_tgt1, np.float32).T] * 2))
    a2p = np.ascontiguousarray(
        np.stack([np.asarray(a_src2, np.float32).ravel(),
                  np.asarray(a_tgt2, np.float32).ravel()], axis=1))
    xT = [np.ascontiguousarray(x[b].T) for b in range(BS)]
    xTb = [t.astype(BF16NP) for t in xT]
    adjTb = [np.ascontiguousarray(adj[b].T.astype(BF16NP)) for b in range(BS)]
    in_maps = []
    for c in range(NCORES):
        b, r = c // 4, c % 4
        sl = slice(r * RB, (r + 1) * RB)
        xrT = np.ascontiguousarray(xT[b][:, sl])
        in_maps.append({
            "adjT": np.ascontiguousarray(adjTb[b][:, sl]),
            "xT": xT[b], "xTb": xTb[b],
            "xrT": xrT, "xrTb": xrT.astype(BF16NP),
            "w1n": W1, "w1Tb": w1Tb, "ws1Tb": ws1Tb,
            "a1s2": a1s2, "a1t2": a1t2,
            "b1": np.asarray(b1, np.float32),
            "w2n": W2, "w2Tb": w2Tb, "ws2Tb": ws2Tb,
            "a2p": a2p,
            "b2": np.asarray(b2, np.float32),
        })
    return in_maps


def kernel(x, adj, W1, a_src1, a_tgt1, Wskip1, b1, W2, a_src2, a_tgt2,
           Wskip2, b2):
    nc = _get_nc()
    in_maps = make_in_maps(x, adj, W1, a_src1, a_tgt1, Wskip1, b1, W2,
                           a_src2, a_tgt2, Wskip2, b2)
    res = run_bass_kernel_spmd(nc, in_maps, core_ids=list(range(NCORES)))
    out = np.empty((BS, N, F1), np.float32)
    for c in range(NCORES):
        b, r = c // 4, c % 4
        out[b, r * RB:(r + 1) * RB, :] = res.results[c]["outT"].T
    return out


# revision 39
# speedup vs baseline: 1.0195x; 1.0124x over previous
"""GAT (2-layer graph attention) Trainium2 Bass kernel, 8-core SPMD.

Sharding: data-parallel over batch (2) x row-blocks (4) -> 8 cores.
Core c handles batch b=c//4, output rows R=[512*(c%4), 512*(c%4+1)).

Key algebra: with z = s_src[i]+s_tgt[j], the GAT edge weight
exp(leaky_relu(z, 0.2)) = max(exp(z), exp(0.2 z)).  Softmax rows are
invariant to a per-row scale, so dividing row i by exp(s_src[i]) gives
unnormalized weights F[j,i] = adj[i,j] * D[j] * max(W[j], g[i]) with
  W[j] = exp(0.8 s_tgt[j]),  D[j] = exp(0.2 s_tgt[j]),  g[i] = exp(-0.8 s_src[i])
-- no per-element transcendentals.  A ones-column in the stationary
operand makes the softmax denominator fall out as a matmul row.

Host prep does layout only (transpose / cast / slice, no math): adj
arrives pre-transposed per core as bf16, x transposed in f32 (scores)
and bf16 (projection), weights pre-transposed in bf16.  Inner work per
(head, j-tile) is one tensor_scalar (4x DVE mode) + one 4-tile-merged
tensor_tensor feeding the TensorE numerator matmul in bf16.

Layer-1 outputs are exchanged within each batch group of 4 cores via a
single AllGather of (proj2^T | s_tgt2).  Work that does not need the
gather is emitted before the collective (tile serializes around it).
"""

import os
import sys

for _p in ("/opt/trn_rl_repo", "/root/.axon_site/_ro/trn_rl_repo"):
    if os.path.isdir(_p) and _p not in sys.path:
        sys.path.insert(0, _p)

import numpy as np
import ml_dtypes

import concourse.bass as bass
import concourse.bacc as bacc
import concourse.mybir as mybir
from concourse import tile
from concourse.bass_utils import run_bass_kernel_spmd

F32 = mybir.dt.float32
BF16 = mybir.dt.bfloat16
AF = mybir.ActivationFunctionType
ALU = mybir.AluOpType
BF16NP = ml_dtypes.bfloat16

BS, N, FIN = 2, 2048, 128
H1, F1 = 8, 64
RB = 512          # row block per core
NJT = N // 128    # 16 j-tiles
NCORES = 8
GROUPS = [[0, 1, 2, 3], [4, 5, 6, 7]]

# layer-1 inner-loop engine split (per head): 16 TSPs, 4 merged TTs
_TSP_POOL1 = {1, 3, 6, 9, 11, 14, 15}   # 7 TSPs per head on Pool
_TT_POOL1 = {3}                          # TT group 3 on Pool, 0-2 DVE
# layer-2: 16 TSPs (4 on ACT via relu/affine pair), 4 merged TTs
_TSP_POOL2 = {1, 3, 5, 9, 13}
_TSP_ACT2 = {7, 15}
_TT_POOL2 = {1, 3}


def build_nc():
    nc = bacc.Bacc("TRN2", target_bir_lowering=False, debug=False,
                   num_devices=NCORES)

    # ---- per-core DRAM I/O (host gives transposed / bf16 layouts) ----
    d_adjT = nc.declare_dram_parameter("adjT", [N, RB], BF16, isOutput=False)
    d_xT = nc.declare_dram_parameter("xT", [FIN, N], F32, isOutput=False)
    d_xTb = nc.declare_dram_parameter("xTb", [FIN, N], BF16, isOutput=False)
    d_xrT = nc.declare_dram_parameter("xrT", [FIN, RB], F32, isOutput=False)
    d_xrTb = nc.declare_dram_parameter("xrTb", [FIN, RB], BF16, isOutput=False)
    d_w1n = nc.declare_dram_parameter("w1n", [H1 * F1, FIN], F32, isOutput=False)
    d_w1Tb = nc.declare_dram_parameter("w1Tb", [FIN, H1 * F1], BF16,
                                       isOutput=False)
    d_ws1Tb = nc.declare_dram_parameter("ws1Tb", [FIN, H1 * F1], BF16,
                                        isOutput=False)
    d_a1s2 = nc.declare_dram_parameter("a1s2", [128, H1], F32, isOutput=False)
    d_a1t2 = nc.declare_dram_parameter("a1t2", [128, H1], F32, isOutput=False)
    d_b1 = nc.declare_dram_parameter("b1", [H1 * F1], F32, isOutput=False)
    d_w2n = nc.declare_dram_parameter("w2n", [F1, H1 * F1], F32, isOutput=False)
    d_w2Tb = nc.declare_dram_parameter("w2Tb", [H1 * F1, F1], BF16,
                                       isOutput=False)
    d_ws2Tb = nc.declare_dram_parameter("ws2Tb", [H1 * F1, F1], BF16,
                                        isOutput=False)
    d_a2p = nc.declare_dram_parameter("a2p", [F1, 2], F32, isOutput=False)
    d_b2 = nc.declare_dram_parameter("b2", [F1], F32, isOutput=False)
    # output: transposed row-block out^T [64, 512] (host transposes back)
    d_out = nc.declare_dram_parameter("outT", [F1, RB], F32, isOutput=True)

    with tile.TileContext(nc) as tc:
        with (
            tc.tile_pool(name="persist", bufs=1) as P,
            tc.tile_pool(name="work", bufs=4) as WK,
            tc.tile_pool(name="gbp", bufs=3) as GB,
            tc.tile_pool(name="ps", bufs=3, space="PSUM") as PS,
            tc.tile_pool(name="psb", bufs=2, space="PSUM") as PSB,
            tc.tile_pool(name="psnum", bufs=3, space="PSUM") as PSN,
            tc.tile_pool(name="dram", bufs=1, space="DRAM") as DR,
        ):
            # ============ loads (emission order ~= DMA priority) ============
            # big/early transfers all on the otherwise-idle SP queue so the
            # ACT/Pool sequencers stay free for compute
            w1n = P.tile([128, 4, FIN], F32, tag="w1n")
            nc.sync.dma_start(w1n[:], d_w1n.rearrange("(k p) c -> p k c", p=128))
            a1sT = P.tile([128, H1], F32, tag="a1sT")
            nc.scalar.dma_start(a1sT[:], d_a1s2[:, :])
            a1tT = P.tile([128, H1], F32, tag="a1tT")
            nc.gpsimd.dma_start(a1tT[:], d_a1t2[:, :])
            xrT = P.tile([128, RB], F32, tag="xrT")
            nc.sync.dma_start(xrT[:], d_xrT[:, :])

            xT = P.tile([128, N], F32, tag="xT")
            nc.sync.dma_start(xT[:, 0:512], d_xT[:, 0:512])
            w1Tb = P.tile([128, H1 * F1], BF16, tag="w1Tb")
            nc.gpsimd.dma_start(w1Tb[:], d_w1Tb[:, :])
            adjT = P.tile([128, NJT, RB], BF16, tag="adjT")
            adjT_src = d_adjT.rearrange("(t p) i -> p t i", p=128)
            nc.sync.dma_start(adjT[:, 0:4, :], adjT_src[:, 0:4, :])
            xTb = P.tile([128, N], BF16, tag="xTb")
            nc.scalar.dma_start(xTb[:, 0:512], d_xTb[:, 0:512])
            nc.sync.dma_start(xT[:, 512:1024], d_xT[:, 512:1024])
            nc.scalar.dma_start(xTb[:, 512:2048], d_xTb[:, 512:2048])
            nc.sync.dma_start(adjT[:, 4:8, :], adjT_src[:, 4:8, :])
            nc.sync.dma_start(xT[:, 1024:2048], d_xT[:, 1024:2048])
            xrTb = P.tile([128, RB], BF16, tag="xrTb")
            nc.gpsimd.dma_start(xrTb[:], d_xrTb[:, :])
            ws1Tb = P.tile([128, H1 * F1], BF16, tag="ws1Tb")
            nc.gpsimd.dma_start(ws1Tb[:], d_ws1Tb[:, :])
            nc.sync.dma_start(adjT[:, 8:12, :], adjT_src[:, 8:12, :])
            b1f = P.tile([128, 4], F32, tag="b1f")
            nc.gpsimd.dma_start(b1f[:], d_b1.rearrange("(k p) -> p k", p=128))
            nc.sync.dma_start(adjT[:, 12:16, :], adjT_src[:, 12:16, :])
            w2n = P.tile([F1, H1 * F1], F32, tag="w2n")
            nc.gpsimd.dma_start(w2n[:], d_w2n[:, :])
            w2Tb = P.tile([128, 4, F1], BF16, tag="w2Tb")
            nc.gpsimd.dma_start(w2Tb[:], d_w2Tb.rearrange("(k p) f -> p k f", p=128))
            ws2Tb = P.tile([128, 4, F1], BF16, tag="ws2Tb")
            nc.gpsimd.dma_start(ws2Tb[:], d_ws2Tb.rearrange("(k p) f -> p k f", p=128))
            a2p = P.tile([F1, 2], F32, tag="a2p")
            nc.gpsimd.dma_start(a2p[:], d_a2p[:, :])
            b2f = P.tile([F1, 1], F32, tag="b2f")
            nc.gpsimd.dma_start(b2f[:], d_b2.ap().rearrange("(f o) -> f o", o=1))

            # constants
            ones1b = P.tile([1, 128], BF16, tag="ones1b")
            nc.vector.memset(ones1b[:], 1.0)
            onesf1 = P.tile([1, F1], F32, tag="onesf1")
            nc.vector.memset(onesf1[:], 1.0)
            ones16 = P.tile([16, 128], BF16, tag="ones16")
            nc.vector.memset(ones16[:], 1.0)
            sel = P.tile([16, H1 * 128], BF16, tag="sel")
            for h in range(H1):
                nc.gpsimd.affine_select(sel[:, h * 128:(h + 1) * 128],
                                        ones16[:], [[0, 128]], ALU.is_equal,
                                        0.0, base=-h, channel_multiplier=1)


            # ============ small exact fp32 matmuls ==========================
            # w1tilde [c=128, 16]: col h = W1_h^T a_src1[h], col 8+h tgt
            ps_wt = PS.tile([128, 512], F32, tag="ps")
            for h in range(H1):
                kt, pr = (h * F1) // 128, (h * F1) % 128
                w1slc = w1n[pr:pr + F1, kt, :]
                nc.tensor.matmul(ps_wt[0:128, h:h + 1], w1slc,
                                 a1sT[pr:pr + F1, h:h + 1])
                nc.tensor.matmul(ps_wt[0:128, 8 + h:9 + h], w1slc,
                                 a1tT[pr:pr + F1, h:h + 1])
            w1t = P.tile([128, 16], F32, tag="w1t")
            nc.vector.tensor_copy(w1t[:], ps_wt[0:128, 0:16])

            # s_src rows for our block -> g (bf16) [16, 512]
            ps_s1r = PS.tile([128, 512], F32, tag="ps")
            nc.tensor.matmul(ps_s1r[0:16, 0:RB], w1t[:], xrT[:])
            g1b = P.tile([16, RB], BF16, tag="g1b")
            nc.scalar.activation(g1b[:], ps_s1r[0:16, 0:RB], AF.Exp, scale=-0.8)

            # S1T [j(128 x 16 chunks), 16] = x @ w1tilde; exp tables per chunk
            ps_s1t = PS.tile([128, 512], F32, tag="ps")
            Wvf = P.tile([128, NJT * 16], F32, tag="Wvf")
            Dvf = P.tile([128, NJT * 16], F32, tag="Dvf")
            for cc in range(4):
                for jc in range(cc * 4, cc * 4 + 4):
                    nc.tensor.matmul(ps_s1t[0:128, jc * 16:(jc + 1) * 16],
                                     xT[:, jc * 128:(jc + 1) * 128], w1t[:])
                cs = slice(cc * 64, (cc + 1) * 64)
                nc.scalar.activation(Wvf[:, cs], ps_s1t[0:128, cs],
                                     AF.Exp, scale=0.8)
                nc.scalar.activation(Dvf[:, cs], ps_s1t[0:128, cs],
                                     AF.Exp, scale=0.2)

            # ============ layer-1 skip:  (x_R @ Wskip1^T)^T  ================
            skipTb = P.tile([128, 4, RB], BF16, tag="skipTb")
            for pr in range(4):
                ps_sk = PS.tile([128, 512], F32, tag="ps")
                nc.tensor.matmul(ps_sk[0:128, 0:RB],
                                 ws1Tb[:, pr * 128:(pr + 1) * 128], xrTb[:])
                nc.scalar.activation(skipTb[:, pr, :], ps_sk[0:128, 0:RB],
                                     AF.Identity, bias=b1f[:, pr:pr + 1])

            # ============ proj1 (+ ones col) ================================
            p1e = P.tile([128, NJT, 8 * 66], BF16, tag="p1e")
            nc.vector.memset(
                p1e[:].rearrange("p j (h q) -> p j h q", q=66)[:, :, :, 64:65],
                1.0)
            for jt in range(NJT):
                ps_p = PSB.tile([128, 512], F32, tag="psb")
                nc.tensor.matmul(ps_p[0:128, 0:512],
                                 xTb[:, jt * 128:(jt + 1) * 128], w1Tb[:])
                dst = p1e[:, jt, :].rearrange("p (h q) -> p h q", q=66)
                src = ps_p[0:128, 0:512].rearrange("p (h q) -> p h q", q=64)
                if jt % 2 == 0:
                    nc.vector.tensor_copy(dst[:, :, 0:64], src)
                else:
                    nc.scalar.activation(dst[:, :, 0:64], src, AF.Copy)

            # layer-2 w2tilde (weights only; do early)
            ps_w2 = PS.tile([128, 512], F32, tag="ps")
            for kt in range(4):
                nc.tensor.matmul(ps_w2[0:128, kt * 2:kt * 2 + 2],
                                 w2n[:, kt * 128:(kt + 1) * 128], a2p[:],
                                 start=True, stop=True)
            w2tb = P.tile([128, 8], BF16, tag="w2tb")
            nc.vector.tensor_copy(w2tb[:], ps_w2[0:128, 0:8])

            # ============ layer-1 head loop =================================
            numb = P.tile([128, 4, RB], BF16, tag="numb")
            den_pairs = []
            for h in range(H1):
                ps_g = PS.tile([128, 512], F32, tag="ps")
                nc.tensor.matmul(ps_g[0:128, 0:RB],
                                 sel[:, h * 128:(h + 1) * 128], g1b[:])
                gbh = GB.tile([128, RB], BF16, tag="gb")
                nc.scalar.activation(gbh[:], ps_g[0:128, 0:RB], AF.Copy)

                if h % 2 == 0:
                    den_pair = P.tile([1, 2, RB], F32, tag=f"den{h // 2}")
                    den_pairs.append(den_pair)
                numT = PSN.tile([65, 512], F32, tag="numT")
                for grp in range(4):
                    q = WK.tile([128, 4, RB], BF16, tag="q")
                    Ft = WK.tile([128, 4, RB], BF16, tag="F")
                    for k in range(4):
                        jt = grp * 4 + k
                        col = slice(jt * 16 + 8 + h, jt * 16 + 9 + h)
                        teng = nc.gpsimd if jt in _TSP_POOL1 else nc.vector
                        teng.tensor_scalar(q[:, k, :], gbh[:], Wvf[:, col],
                                           Dvf[:, col], ALU.max, ALU.mult)
                    geng = nc.gpsimd if grp in _TT_POOL1 else nc.vector
                    geng.tensor_tensor(Ft[:], q[:],
                                       adjT[:, grp * 4:grp * 4 + 4, :], ALU.mult)
                    for k in range(4):
                        jt = grp * 4 + k
                        nc.tensor.matmul(numT[0:65, 0:RB],
                                         p1e[:, jt, h * 66:h * 66 + 65],
                                         Ft[:, k, :],
                                         start=(jt == 0), stop=(jt == NJT - 1))
                nc.scalar.activation(den_pairs[h // 2][0:1, h % 2, :],
                                     numT[64:65, 0:RB], AF.Copy)
                nc.scalar.activation(numb[(h % 2) * 64:(h % 2) * 64 + 64, h // 2, :],
                                     numT[0:64, 0:RB], AF.Copy)

            # h_out^T = elu(num/den + (skip + b1)), kept bf16, per pair so
            # late pairs overlap earlier heads' compute
            houtb = P.tile([128, 4, RB], BF16, tag="houtb")
            for pr in range(4):
                rec_p = P.tile([1, 2, RB], F32, tag=f"rec{pr}")
                nc.vector.reciprocal_approx_fast(rec_p[:], den_pairs[pr][:])
                rdb = GB.tile([128, RB], BF16, tag="gb")
                ps_r = PS.tile([128, 512], F32, tag="ps")
                nc.tensor.matmul(ps_r[0:64, 0:RB], onesf1[:], rec_p[0:1, 0, :])
                nc.tensor.matmul(ps_r[64:128, 0:RB], onesf1[:], rec_p[0:1, 1, :])
                nc.scalar.activation(rdb[:], ps_r[0:128, 0:RB], AF.Copy)
                hpre = WK.tile([128, RB], BF16, tag="hpre")
                heng = nc.gpsimd if pr % 2 == 0 else nc.vector
                heng.tensor_mul(hpre[:], numb[:, pr, :], rdb[:])
                u = WK.tile([128, RB], BF16, tag="u")
                heng.tensor_add(u[:], hpre[:], skipTb[:, pr, :])
                m0 = WK.tile([128, RB], BF16, tag="hpre")
                nc.vector.tensor_scalar(m0[:], u[:], 0.0, None, ALU.min)
                e = WK.tile([128, RB], BF16, tag="e")
                nc.scalar.activation(e[:], m0[:], AF.Exp)
                nc.vector.scalar_tensor_tensor(
                    houtb[:, pr, :], e[:], -1.0, u[:], ALU.add, ALU.max)

            # ============ layer-2 local pieces ==============================
            # S2: s_src2 -> psum row 0, s_tgt2 -> psum row 32
            ps_s2 = PS.tile([128, 512], F32, tag="ps")
            for kt in range(4):
                nc.tensor.matmul(ps_s2[0:1, 0:RB], w2tb[:, kt * 2:kt * 2 + 1],
                                 houtb[:, kt, :], start=(kt == 0), stop=(kt == 3))
            for kt in range(4):
                nc.tensor.matmul(ps_s2[32:33, 0:RB], w2tb[:, kt * 2 + 1:kt * 2 + 2],
                                 houtb[:, kt, :], start=(kt == 0), stop=(kt == 3))
            g2row = P.tile([1, RB], BF16, tag="g2row")
            nc.scalar.activation(g2row[:], ps_s2[0:1, 0:RB], AF.Exp, scale=-0.8)
            stg2b = P.tile([1, RB], BF16, tag="stg2b")
            nc.scalar.activation(stg2b[:], ps_s2[32:33, 0:RB], AF.Copy)

            # proj2^T local [64, 512] in bf16 for the gather
            ps_p2 = PS.tile([128, 512], F32, tag="ps")
            for kt in range(4):
                nc.tensor.matmul(ps_p2[0:64, 0:RB], w2Tb[:, kt, :],
                                 houtb[:, kt, :], start=(kt == 0), stop=(kt == 3))
            p2Tb = P.tile([F1, RB], BF16, tag="p2Tb")
            nc.scalar.activation(p2Tb[:], ps_p2[0:64, 0:RB], AF.Copy)

            # ---- gather-independent layer-2 prep, before the collective ----
            p2e = P.tile([128, NJT, F1 + 1], BF16, tag="p2e")
            ps_g2 = PS.tile([128, 512], F32, tag="ps")
            nc.tensor.matmul(ps_g2[0:128, 0:RB], ones1b[:], g2row[:])
            g2bc = GB.tile([128, RB], BF16, tag="gb")
            nc.scalar.activation(g2bc[:], ps_g2[0:128, 0:RB], AF.Copy)
            ps_sk2 = PS.tile([128, 512], F32, tag="ps")
            for kt in range(4):
                nc.tensor.matmul(ps_sk2[0:64, 0:RB], ws2Tb[:, kt, :],
                                 houtb[:, kt, :], start=(kt == 0), stop=(kt == 3))

            # ============ AllGather within batch group ======================
            # gin bf16 [4*65, 128]: rows (s, 0..63) = proj2^T slices,
            # row (s, 64) = s_tgt2
            gin = DR.tile([4 * (F1 + 1), 128], BF16)
            gin_v = gin.rearrange("(s f) p -> s f p", f=F1 + 1)
            nc.sync.dma_start(
                gin_v[:, 0:F1, :].rearrange("s f p -> f s p"),
                p2Tb[:].rearrange("f (s p) -> f s p", p=128))
            nc.scalar.dma_start(
                gin_v[:, F1:F1 + 1, :].rearrange("s o p -> o s p"),
                stg2b[:].rearrange("o (s p) -> o s p", p=128))
            gout = DR.tile([4 * 4 * (F1 + 1), 128], BF16)
            nc.gpsimd.collective_compute(
                "AllGather", ALU.bypass, replica_groups=GROUPS,
                ins=[gin.opt()], outs=[gout.opt()])
            gout_v = gout.rearrange("(c s f) p -> c s f p", s=4, f=F1 + 1)

            # ============ layer-2 attention =================================
            nc.sync.dma_start(
                p2e[:],
                gout_v.rearrange("c s f p -> p (c s) f"))
            st2Tb = P.tile([128, 4, 4], BF16, tag="st2Tb")
            nc.scalar.dma_start(
                st2Tb[:], gout_v[:, :, F1, :].rearrange("c s p -> p c s"))
            nc.vector.memset(p2e[:, :, F1:F1 + 1], 1.0)  # denominator column
            W2vf = P.tile([128, 16], F32, tag="W2vf")
            nc.scalar.activation(W2vf[:], st2Tb[:].rearrange("p c s -> p (c s)"),
                                 AF.Exp, scale=0.8)
            D2v = P.tile([128, 16], F32, tag="D2v")
            nc.scalar.activation(D2v[:], st2Tb[:].rearrange("p c s -> p (c s)"),
                                 AF.Exp, scale=0.2)
            # tables for the ACT-side q ops: q = (relu(g - W) * D + W*D)
            W2neg = P.tile([128, 16], F32, tag="W2neg")
            nc.vector.tensor_scalar(W2neg[:], W2vf[:], -1.0, None, ALU.mult)
            E2v = P.tile([128, 16], F32, tag="E2v")
            nc.vector.tensor_tensor(E2v[:], W2vf[:], D2v[:], ALU.mult)

            numT2 = PSN.tile([65, 512], F32, tag="numT")
            for grp in range(4):
                q2 = WK.tile([128, 4, RB], BF16, tag="q")
                F2t = WK.tile([128, 4, RB], BF16, tag="F")
                for k in range(4):
                    jt = grp * 4 + k
                    col = slice(jt, jt + 1)
                    if jt in _TSP_ACT2:
                        r2 = WK.tile([128, RB], BF16, tag="r2")
                        nc.scalar.activation(r2[:], g2bc[:], AF.Relu,
                                             bias=W2neg[:, col])
                        nc.scalar.activation(q2[:, k, :], r2[:], AF.Identity,
                                             scale=D2v[:, col],
                                             bias=E2v[:, col])
                    else:
                        teng = nc.gpsimd if jt in _TSP_POOL2 else nc.vector
                        teng.tensor_scalar(q2[:, k, :], g2bc[:], W2vf[:, col],
                                           D2v[:, col], ALU.max, ALU.mult)
                geng = nc.gpsimd if grp in _TT_POOL2 else nc.vector
                geng.tensor_tensor(F2t[:], q2[:],
                                   adjT[:, grp * 4:grp * 4 + 4, :], ALU.mult)
                for k in range(4):
                    jt = grp * 4 + k
                    nc.tensor.matmul(numT2[0:F1 + 1, 0:RB],
                                     p2e[:, jt, 0:F1 + 1], F2t[:, k, :],
                                     start=(jt == 0), stop=(jt == NJT - 1))

            den2 = P.tile([1, RB], F32, tag="den2")
            nc.scalar.activation(den2[:], numT2[F1:F1 + 1, 0:RB], AF.Copy)
            rec2 = P.tile([1, RB], F32, tag="rec2")
            nc.vector.reciprocal_approx_fast(rec2[:], den2[:])
            ps_r2 = PS.tile([128, 512], F32, tag="ps")
            nc.tensor.matmul(ps_r2[0:64, 0:RB], onesf1[:], rec2[:])
            rdb2 = GB.tile([128, RB], BF16, tag="rdb")
            nc.scalar.activation(rdb2[0:64, :], ps_r2[0:64, 0:RB], AF.Copy)

            t2 = WK.tile([F1, RB], F32, tag="t2")
            nc.vector.tensor_mul(t2[:], numT2[0:F1, 0:RB], rdb2[0:64, :])
            o2 = WK.tile([F1, RB], F32, tag="o2")
            nc.vector.scalar_tensor_tensor(
                o2[:], t2[:], b2f[:], ps_sk2[0:64, 0:RB], ALU.add, ALU.add)
            nc.sync.dma_start(d_out[:, :], o2[:])

    nc.compile()
    return nc


_NC_CACHE = None


def _get_nc():
    global _NC_CACHE
    if _NC_CACHE is None:
        _NC_CACHE = build_nc()
    return _NC_CACHE


def make_in_maps(x, adj, W1, a_src1, a_tgt1, Wskip1, b1, W2, a_src2, a_tgt2,
                 Wskip2, b2):
    x = np.asarray(x, np.float32)
    adj = np.asarray(adj, np.float32)
    W1 = np.asarray(W1, np.float32)
    W2 = np.asarray(W2, np.float32)
    Wskip1 = np.asarray(Wskip1, np.float32)
    Wskip2 = np.asarray(Wskip2, np.float32)
    w1Tb = np.ascontiguousarray(W1.T).astype(BF16NP)
    ws1Tb = np.ascontiguousarray(Wskip1.T).astype(BF16NP)
    w2Tb = np.ascontiguousarray(W2.T).astype(BF16NP)
    ws2Tb = np.ascontiguousarray(Wskip2.T).astype(BF16NP)
    a1s2 = np.ascontiguousarray(
        np.vstack([np.asarray(a_src1, np.float32).T] * 2))
    a1t2 = np.ascontiguousarray(
        np.vstack([np.asarray(a_tgt1, np.float32).T] * 2))
    a2p = np.ascontiguousarray(
        np.stack([np.asarray(a_src2, np.float32).ravel(),
                  np.asarray(a_tgt2, np.float32).ravel()], axis=1))
    xT = [np.ascontiguousarray(x[b].T) for b in range(BS)]
    xTb = [t.astype(BF16NP) for t in xT]
    adjTb = [np.ascontiguousarray(adj[b].T.astype(BF16NP)) for b in range(BS)]
    in_maps = []
    for c in range(NCORES):
        b, r = c // 4, c % 4
        sl = slice(r * RB, (r + 1) * RB)
        xrT = np.ascontiguousarray(xT[b][:, sl])
        in_maps.append({
            "adjT": np.ascontiguousarray(adjTb[b][:, sl]),
            "xT": xT[b], "xTb": xTb[b],
            "xrT": xrT, "xrTb": xrT.astype(BF16NP),
            "w1n": W1, "w1Tb": w1Tb, "ws1Tb": ws1Tb,
            "a1s2": a1s2, "a1t2": a1t2,
            "b1": np.asarray(b1, np.float32),
            "w2n": W2, "w2Tb": w2Tb, "ws2Tb": ws2Tb,
            "a2p": a2p,
            "b2": np.asarray(b2, np.float32),
        })
    return in_maps


def kernel(x, adj, W1, a_src1, a_tgt1, Wskip1, b1, W2, a_src2, a_tgt2,
           Wskip2, b2):
    nc = _get_nc()
    in_maps = make_in_maps(x, adj, W1, a_src1, a_tgt1, Wskip1, b1, W2,
                           a_src2, a_tgt2, Wskip2, b2)
    res = run_bass_kernel_spmd(nc, in_maps, core_ids=list(range(NCORES)))
    out = np.empty((BS, N, F1), np.float32)
    for c in range(NCORES):
        b, r = c // 4, c % 4
        out[b, r * RB:(r + 1) * RB, :] = res.results[c]["outT"].T
    return out


# revision 40
# speedup vs baseline: 1.0199x; 1.0003x over previous
"""GAT (2-layer graph attention) Trainium2 Bass kernel, 8-core SPMD.

Sharding: data-parallel over batch (2) x row-blocks (4) -> 8 cores.
Core c handles batch b=c//4, output rows R=[512*(c%4), 512*(c%4+1)).

Key algebra: with z = s_src[i]+s_tgt[j], the GAT edge weight
exp(leaky_relu(z, 0.2)) = max(exp(z), exp(0.2 z)).  Softmax rows are
invariant to a per-row scale, so dividing row i by exp(s_src[i]) gives
unnormalized weights F[j,i] = adj[i,j] * D[j] * max(W[j], g[i]) with
  W[j] = exp(0.8 s_tgt[j]),  D[j] = exp(0.2 s_tgt[j]),  g[i] = exp(-0.8 s_src[i])
-- no per-element transcendentals.  A ones-column in the stationary
operand makes the softmax denominator fall out as a matmul row.

Host prep does layout only (transpose / cast / slice, no math): adj
arrives pre-transposed per core as bf16, x transposed in f32 (scores)
and bf16 (projection), weights pre-transposed in bf16.  Inner work per
(head, j-tile) is one tensor_scalar (4x DVE mode) + one 4-tile-merged
tensor_tensor feeding the TensorE numerator matmul in bf16.

Layer-1 outputs are exchanged within each batch group of 4 cores via a
single AllGather of (proj2^T | s_tgt2).  Work that does not need the
gather is emitted before the collective (tile serializes around it).
"""

import os
import sys

for _p in ("/opt/trn_rl_repo", "/root/.axon_site/_ro/trn_rl_repo"):
    if os.path.isdir(_p) and _p not in sys.path:
        sys.path.insert(0, _p)

import numpy as np
import ml_dtypes

import concourse.bass as bass
import concourse.bacc as bacc
import concourse.mybir as mybir
from concourse import tile
from concourse.bass_utils import run_bass_kernel_spmd

F32 = mybir.dt.float32
BF16 = mybir.dt.bfloat16
AF = mybir.ActivationFunctionType
ALU = mybir.AluOpType
BF16NP = ml_dtypes.bfloat16

BS, N, FIN = 2, 2048, 128
H1, F1 = 8, 64
RB = 512          # row block per core
NJT = N // 128    # 16 j-tiles
NCORES = 8
GROUPS = [[0, 1, 2, 3], [4, 5, 6, 7]]

# layer-1 inner-loop engine split (per head): 16 TSPs, 4 merged TTs
_TSP_POOL1 = {1, 3, 6, 9, 11, 14, 15}   # 7 TSPs per head on Pool
_TT_POOL1 = {3}                          # TT group 3 on Pool, 0-2 DVE
# layer-2: 16 TSPs (4 on ACT via relu/affine pair), 4 merged TTs
_TSP_POOL2 = {1, 3, 5, 9, 13}
_TSP_ACT2 = {7, 15}
_TT_POOL2 = {1, 3}


def build_nc():
    nc = bacc.Bacc("TRN2", target_bir_lowering=False, debug=False,
                   num_devices=NCORES)

    # ---- per-core DRAM I/O (host gives transposed / bf16 layouts) ----
    d_adjT = nc.declare_dram_parameter("adjT", [N, RB], BF16, isOutput=False)
    d_xT = nc.declare_dram_parameter("xT", [FIN, N], F32, isOutput=False)
    d_xTb = nc.declare_dram_parameter("xTb", [FIN, N], BF16, isOutput=False)
    d_xrT = nc.declare_dram_parameter("xrT", [FIN, RB], F32, isOutput=False)
    d_xrTb = nc.declare_dram_parameter("xrTb", [FIN, RB], BF16, isOutput=False)
    d_w1n = nc.declare_dram_parameter("w1n", [H1 * F1, FIN], F32, isOutput=False)
    d_w1Tb = nc.declare_dram_parameter("w1Tb", [FIN, H1 * F1], BF16,
                                       isOutput=False)
    d_ws1Tb = nc.declare_dram_parameter("ws1Tb", [FIN, H1 * F1], BF16,
                                        isOutput=False)
    d_a1s2 = nc.declare_dram_parameter("a1s2", [128, H1], F32, isOutput=False)
    d_a1t2 = nc.declare_dram_parameter("a1t2", [128, H1], F32, isOutput=False)
    d_b1 = nc.declare_dram_parameter("b1", [H1 * F1], F32, isOutput=False)
    d_w2n = nc.declare_dram_parameter("w2n", [F1, H1 * F1], F32, isOutput=False)
    d_w2Tb = nc.declare_dram_parameter("w2Tb", [H1 * F1, F1], BF16,
                                       isOutput=False)
    d_ws2Tb = nc.declare_dram_parameter("ws2Tb", [H1 * F1, F1], BF16,
                                        isOutput=False)
    d_a2p = nc.declare_dram_parameter("a2p", [F1, 2], F32, isOutput=False)
    d_b2 = nc.declare_dram_parameter("b2", [F1], F32, isOutput=False)
    # output: transposed row-block out^T [64, 512] (host transposes back)
    d_out = nc.declare_dram_parameter("outT", [F1, RB], F32, isOutput=True)

    with tile.TileContext(nc) as tc:
        with (
            tc.tile_pool(name="persist", bufs=1) as P,
            tc.tile_pool(name="work", bufs=4) as WK,
            tc.tile_pool(name="gbp", bufs=3) as GB,
            tc.tile_pool(name="ps", bufs=3, space="PSUM") as PS,
            tc.tile_pool(name="psb", bufs=2, space="PSUM") as PSB,
            tc.tile_pool(name="psnum", bufs=3, space="PSUM") as PSN,
            tc.tile_pool(name="dram", bufs=1, space="DRAM") as DR,
        ):
            # ============ loads (emission order ~= DMA priority) ============
            # big/early transfers all on the otherwise-idle SP queue so the
            # ACT/Pool sequencers stay free for compute
            w1n = P.tile([128, 4, FIN], F32, tag="w1n")
            nc.sync.dma_start(w1n[:], d_w1n.rearrange("(k p) c -> p k c", p=128))
            a1sT = P.tile([128, H1], F32, tag="a1sT")
            nc.scalar.dma_start(a1sT[:], d_a1s2[:, :])
            a1tT = P.tile([128, H1], F32, tag="a1tT")
            nc.gpsimd.dma_start(a1tT[:], d_a1t2[:, :])
            xrT = P.tile([128, RB], F32, tag="xrT")
            nc.sync.dma_start(xrT[:], d_xrT[:, :])

            xT = P.tile([128, N], F32, tag="xT")
            nc.sync.dma_start(xT[:, 0:512], d_xT[:, 0:512])
            w1Tb = P.tile([128, H1 * F1], BF16, tag="w1Tb")
            nc.gpsimd.dma_start(w1Tb[:], d_w1Tb[:, :])
            adjT = P.tile([128, NJT, RB], BF16, tag="adjT")
            adjT_src = d_adjT.rearrange("(t p) i -> p t i", p=128)
            nc.sync.dma_start(adjT[:, 0:4, :], adjT_src[:, 0:4, :])
            xTb = P.tile([128, N], BF16, tag="xTb")
            nc.scalar.dma_start(xTb[:, 0:512], d_xTb[:, 0:512])
            nc.sync.dma_start(xT[:, 512:1024], d_xT[:, 512:1024])
            nc.scalar.dma_start(xTb[:, 512:2048], d_xTb[:, 512:2048])
            nc.sync.dma_start(adjT[:, 4:8, :], adjT_src[:, 4:8, :])
            nc.sync.dma_start(xT[:, 1024:2048], d_xT[:, 1024:2048])
            xrTb = P.tile([128, RB], BF16, tag="xrTb")
            nc.gpsimd.dma_start(xrTb[:], d_xrTb[:, :])
            ws1Tb = P.tile([128, H1 * F1], BF16, tag="ws1Tb")
            nc.gpsimd.dma_start(ws1Tb[:], d_ws1Tb[:, :])
            nc.sync.dma_start(adjT[:, 8:12, :], adjT_src[:, 8:12, :])
            b1f = P.tile([128, 4], F32, tag="b1f")
            nc.gpsimd.dma_start(b1f[:], d_b1.rearrange("(k p) -> p k", p=128))
            nc.sync.dma_start(adjT[:, 12:16, :], adjT_src[:, 12:16, :])
            w2n = P.tile([F1, H1 * F1], F32, tag="w2n")
            nc.gpsimd.dma_start(w2n[:], d_w2n[:, :])
            w2Tb = P.tile([128, 4, F1], BF16, tag="w2Tb")
            nc.gpsimd.dma_start(w2Tb[:], d_w2Tb.rearrange("(k p) f -> p k f", p=128))
            ws2Tb = P.tile([128, 4, F1], BF16, tag="ws2Tb")
            nc.gpsimd.dma_start(ws2Tb[:], d_ws2Tb.rearrange("(k p) f -> p k f", p=128))
            a2p = P.tile([F1, 2], F32, tag="a2p")
            nc.gpsimd.dma_start(a2p[:], d_a2p[:, :])
            b2f = P.tile([F1, 1], F32, tag="b2f")
            nc.gpsimd.dma_start(b2f[:], d_b2.ap().rearrange("(f o) -> f o", o=1))

            # constants
            ones1b = P.tile([1, 128], BF16, tag="ones1b")
            nc.vector.memset(ones1b[:], 1.0)
            onesf1 = P.tile([1, F1], F32, tag="onesf1")
            nc.vector.memset(onesf1[:], 1.0)
            ones16 = P.tile([16, 128], BF16, tag="ones16")
            nc.vector.memset(ones16[:], 1.0)
            sel = P.tile([16, H1 * 128], BF16, tag="sel")
            for h in range(H1):
                nc.gpsimd.affine_select(sel[:, h * 128:(h + 1) * 128],
                                        ones16[:], [[0, 128]], ALU.is_equal,
                                        0.0, base=-h, channel_multiplier=1)


            # ============ small exact fp32 matmuls ==========================
            # w1tilde [c=128, 16]: col h = W1_h^T a_src1[h], col 8+h tgt
            ps_wt = PS.tile([128, 512], F32, tag="ps")
            for h in range(H1):
                kt, pr = (h * F1) // 128, (h * F1) % 128
                w1slc = w1n[pr:pr + F1, kt, :]
                nc.tensor.matmul(ps_wt[0:128, h:h + 1], w1slc,
                                 a1sT[pr:pr + F1, h:h + 1])
                nc.tensor.matmul(ps_wt[0:128, 8 + h:9 + h], w1slc,
                                 a1tT[pr:pr + F1, h:h + 1])
            w1t = P.tile([128, 16], F32, tag="w1t")
            nc.vector.tensor_copy(w1t[:], ps_wt[0:128, 0:16])

            # s_src rows for our block -> g (bf16) [16, 512]
            ps_s1r = PS.tile([128, 512], F32, tag="ps")
            nc.tensor.matmul(ps_s1r[0:16, 0:RB], w1t[:], xrT[:])
            g1b = P.tile([16, RB], BF16, tag="g1b")
            nc.scalar.activation(g1b[:], ps_s1r[0:16, 0:RB], AF.Exp, scale=-0.8)

            # S1T [j(128 x 16 chunks), 16] = x @ w1tilde; exp tables per chunk
            ps_s1t = PS.tile([128, 512], F32, tag="ps")
            Wvf = P.tile([128, NJT * 16], F32, tag="Wvf")
            Dvf = P.tile([128, NJT * 16], F32, tag="Dvf")
            for cc in range(4):
                for jc in range(cc * 4, cc * 4 + 4):
                    nc.tensor.matmul(ps_s1t[0:128, jc * 16:(jc + 1) * 16],
                                     xT[:, jc * 128:(jc + 1) * 128], w1t[:])
                cs = slice(cc * 64, (cc + 1) * 64)
                nc.scalar.activation(Wvf[:, cs], ps_s1t[0:128, cs],
                                     AF.Exp, scale=0.8)
                nc.scalar.activation(Dvf[:, cs], ps_s1t[0:128, cs],
                                     AF.Exp, scale=0.2)

            # ============ layer-1 skip:  (x_R @ Wskip1^T)^T  ================
            skipTb = P.tile([128, 4, RB], BF16, tag="skipTb")
            for pr in range(4):
                ps_sk = PS.tile([128, 512], F32, tag="ps")
                nc.tensor.matmul(ps_sk[0:128, 0:RB],
                                 ws1Tb[:, pr * 128:(pr + 1) * 128], xrTb[:])
                nc.scalar.activation(skipTb[:, pr, :], ps_sk[0:128, 0:RB],
                                     AF.Identity, bias=b1f[:, pr:pr + 1])

            # ============ proj1 (+ ones col) ================================
            p1e = P.tile([128, NJT, 8 * 66], BF16, tag="p1e")
            nc.vector.memset(
                p1e[:].rearrange("p j (h q) -> p j h q", q=66)[:, :, :, 64:65],
                1.0)
            for jt in range(NJT):
                ps_p = PSB.tile([128, 512], F32, tag="psb")
                nc.tensor.matmul(ps_p[0:128, 0:512],
                                 xTb[:, jt * 128:(jt + 1) * 128], w1Tb[:])
                dst = p1e[:, jt, :].rearrange("p (h q) -> p h q", q=66)
                src = ps_p[0:128, 0:512].rearrange("p (h q) -> p h q", q=64)
                if jt % 2 == 0:
                    nc.vector.tensor_copy(dst[:, :, 0:64], src)
                else:
                    nc.scalar.activation(dst[:, :, 0:64], src, AF.Copy)

            # layer-2 w2tilde (weights only; do early)
            ps_w2 = PS.tile([128, 512], F32, tag="ps")
            for kt in range(4):
                nc.tensor.matmul(ps_w2[0:128, kt * 2:kt * 2 + 2],
                                 w2n[:, kt * 128:(kt + 1) * 128], a2p[:],
                                 start=True, stop=True)
            w2tb = P.tile([128, 8], BF16, tag="w2tb")
            nc.vector.tensor_copy(w2tb[:], ps_w2[0:128, 0:8])

            # ============ layer-1 head loop =================================
            numb = P.tile([128, 4, RB], BF16, tag="numb")
            den_pairs = []
            for h in range(H1):
                ps_g = PS.tile([128, 512], F32, tag="ps")
                nc.tensor.matmul(ps_g[0:128, 0:RB],
                                 sel[:, h * 128:(h + 1) * 128], g1b[:])
                gbh = GB.tile([128, RB], BF16, tag="gb")
                nc.scalar.activation(gbh[:], ps_g[0:128, 0:RB], AF.Copy)

                if h % 2 == 0:
                    den_pair = P.tile([1, 2, RB], F32, tag=f"den{h // 2}")
                    den_pairs.append(den_pair)
                numT = PSN.tile([65, 512], F32, tag="numT")
                for grp in range(4):
                    q = WK.tile([128, 4, RB], BF16, tag="q")
                    Ft = WK.tile([128, 4, RB], BF16, tag="F")
                    for k in range(4):
                        jt = grp * 4 + k
                        col = slice(jt * 16 + 8 + h, jt * 16 + 9 + h)
                        teng = nc.gpsimd if jt in _TSP_POOL1 else nc.vector
                        teng.tensor_scalar(q[:, k, :], gbh[:], Wvf[:, col],
                                           Dvf[:, col], ALU.max, ALU.mult)
                    geng = nc.gpsimd if grp in _TT_POOL1 else nc.vector
                    geng.tensor_tensor(Ft[:], q[:],
                                       adjT[:, grp * 4:grp * 4 + 4, :], ALU.mult)
                    for k in range(4):
                        jt = grp * 4 + k
                        nc.tensor.matmul(numT[0:65, 0:RB],
                                         p1e[:, jt, h * 66:h * 66 + 65],
                                         Ft[:, k, :],
                                         start=(jt == 0), stop=(jt == NJT - 1))
                nc.scalar.activation(den_pairs[h // 2][0:1, h % 2, :],
                                     numT[64:65, 0:RB], AF.Copy)
                nc.scalar.activation(numb[(h % 2) * 64:(h % 2) * 64 + 64, h // 2, :],
                                     numT[0:64, 0:RB], AF.Copy)

            # h_out^T = elu(num/den + (skip + b1)), kept bf16, per pair so
            # late pairs overlap earlier heads' compute
            houtb = P.tile([128, 4, RB], BF16, tag="houtb")
            for pr in range(4):
                rec_p = P.tile([1, 2, RB], F32, tag=f"rec{pr}")
                nc.vector.reciprocal_approx_fast(rec_p[:], den_pairs[pr][:])
                rdb = GB.tile([128, RB], BF16, tag="gb")
                ps_r = PS.tile([128, 512], F32, tag="ps")
                nc.tensor.matmul(ps_r[0:64, 0:RB], onesf1[:], rec_p[0:1, 0, :])
                nc.tensor.matmul(ps_r[64:128, 0:RB], onesf1[:], rec_p[0:1, 1, :])
                nc.scalar.activation(rdb[:], ps_r[0:128, 0:RB], AF.Copy)
                hpre = WK.tile([128, RB], BF16, tag="hpre")
                heng = nc.gpsimd if pr % 2 == 0 else nc.vector
                heng.tensor_mul(hpre[:], numb[:, pr, :], rdb[:])
                u = WK.tile([128, RB], BF16, tag="u")
                heng.tensor_add(u[:], hpre[:], skipTb[:, pr, :])
                m0 = WK.tile([128, RB], BF16, tag="hpre")
                nc.vector.tensor_scalar(m0[:], u[:], 0.0, None, ALU.min)
                e = WK.tile([128, RB], BF16, tag="e")
                nc.scalar.activation(e[:], m0[:], AF.Exp)
                nc.vector.scalar_tensor_tensor(
                    houtb[:, pr, :], e[:], -1.0, u[:], ALU.add, ALU.max)

            # ============ layer-2 local pieces ==============================
            # S2: s_src2 -> psum row 0, s_tgt2 -> psum row 32
            ps_s2 = PS.tile([128, 512], F32, tag="ps")
            for kt in range(4):
                nc.tensor.matmul(ps_s2[0:1, 0:RB], w2tb[:, kt * 2:kt * 2 + 1],
                                 houtb[:, kt, :], start=(kt == 0), stop=(kt == 3))
            for kt in range(4):
                nc.tensor.matmul(ps_s2[32:33, 0:RB], w2tb[:, kt * 2 + 1:kt * 2 + 2],
                                 houtb[:, kt, :], start=(kt == 0), stop=(kt == 3))
            g2row = P.tile([1, RB], BF16, tag="g2row")
            nc.scalar.activation(g2row[:], ps_s2[0:1, 0:RB], AF.Exp, scale=-0.8)
            stg2b = P.tile([1, RB], BF16, tag="stg2b")
            nc.scalar.activation(stg2b[:], ps_s2[32:33, 0:RB], AF.Copy)

            # proj2^T local [64, 512] in bf16 for the gather
            ps_p2 = PS.tile([128, 512], F32, tag="ps")
            for kt in range(4):
                nc.tensor.matmul(ps_p2[0:64, 0:RB], w2Tb[:, kt, :],
                                 houtb[:, kt, :], start=(kt == 0), stop=(kt == 3))
            p2Tb = P.tile([F1, RB], BF16, tag="p2Tb")
            nc.scalar.activation(p2Tb[:], ps_p2[0:64, 0:RB], AF.Copy)

            # ---- gather-independent layer-2 prep, before the collective ----
            p2e = P.tile([128, NJT, F1 + 1], BF16, tag="p2e")
            ps_g2 = PS.tile([128, 512], F32, tag="ps")
            nc.tensor.matmul(ps_g2[0:128, 0:RB], ones1b[:], g2row[:])
            g2bc = GB.tile([128, RB], BF16, tag="gb")
            nc.scalar.activation(g2bc[:], ps_g2[0:128, 0:RB], AF.Copy)
            ps_sk2 = PS.tile([128, 512], F32, tag="ps")
            for kt in range(4):
                nc.tensor.matmul(ps_sk2[0:64, 0:RB], ws2Tb[:, kt, :],
                                 houtb[:, kt, :], start=(kt == 0), stop=(kt == 3))

            # ============ AllGather within batch group ======================
            # gin bf16 [4*65, 128]: rows (s, 0..63) = proj2^T slices,
            # row (s, 64) = s_tgt2
            gin = DR.tile([4 * (F1 + 1), 128], BF16)
            gin_v = gin.rearrange("(s f) p -> s f p", f=F1 + 1)
            nc.sync.dma_start(
                gin_v[:, 0:F1, :].rearrange("s f p -> f s p"),
                p2Tb[:].rearrange("f (s p) -> f s p", p=128))
            nc.scalar.dma_start(
                gin_v[:, F1:F1 + 1, :].rearrange("s o p -> o s p"),
                stg2b[:].rearrange("o (s p) -> o s p", p=128))
            gout = DR.tile([4 * 4 * (F1 + 1), 128], BF16)
            nc.gpsimd.collective_compute(
                "AllGather", ALU.bypass, replica_groups=GROUPS,
                ins=[gin.opt()], outs=[gout.opt()])
            gout_v = gout.rearrange("(c s f) p -> c s f p", s=4, f=F1 + 1)

            # ============ layer-2 attention =================================
            nc.sync.dma_start(
                p2e[:],
                gout_v.rearrange("c s f p -> p (c s) f"))
            st2Tb = P.tile([128, 4, 4], BF16, tag="st2Tb")
            nc.scalar.dma_start(
                st2Tb[:], gout_v[:, :, F1, :].rearrange("c s p -> p c s"))
            nc.vector.memset(p2e[:, :, F1:F1 + 1], 1.0)  # denominator column
            W2vf = P.tile([128, 16], F32, tag="W2vf")
            nc.scalar.activation(W2vf[:], st2Tb[:].rearrange("p c s -> p (c s)"),
                                 AF.Exp, scale=0.8)
            D2v = P.tile([128, 16], F32, tag="D2v")
            nc.scalar.activation(D2v[:], st2Tb[:].rearrange("p c s -> p (c s)"),
                                 AF.Exp, scale=0.2)
            # tables for the ACT-side q ops: q = (relu(g - W) * D + W*D)
            W2neg = P.tile([128, 16], F32, tag="W2neg")
            nc.vector.tensor_scalar(W2neg[:], W2vf[:], -1.0, None, ALU.mult)
            E2v = P.tile([128, 16], F32, tag="E2v")
            nc.vector.tensor_tensor(E2v[:], W2vf[:], D2v[:], ALU.mult)

            numT2 = PSN.tile([65, 512], F32, tag="numT")
            for grp in range(4):
                q2 = WK.tile([128, 4, RB], BF16, tag="q")
                F2t = WK.tile([128, 4, RB], BF16, tag="F")
                for k in range(4):
                    jt = grp * 4 + k
                    col = slice(jt, jt + 1)
                    if jt in _TSP_ACT2:
                        r2 = WK.tile([128, RB], BF16, tag="r2")
                        nc.scalar.activation(r2[:], g2bc[:], AF.Relu,
                                             bias=W2neg[:, col])
                        nc.scalar.activation(q2[:, k, :], r2[:], AF.Identity,
                                             scale=D2v[:, col],
                                             bias=E2v[:, col])
                    else:
                        teng = nc.gpsimd if jt in _TSP_POOL2 else nc.vector
                        teng.tensor_scalar(q2[:, k, :], g2bc[:], W2vf[:, col],
                                           D2v[:, col], ALU.max, ALU.mult)
                geng = nc.gpsimd if grp in _TT_POOL2 else nc.vector
                geng.tensor_tensor(F2t[:], q2[:],
                                   adjT[:, grp * 4:grp * 4 + 4, :], ALU.mult)
                for k in range(4):
                    jt = grp * 4 + k
                    nc.tensor.matmul(numT2[0:F1 + 1, 0:RB],
                                     p2e[:, jt, 0:F1 + 1], F2t[:, k, :],
                                     start=(jt == 0), stop=(jt == NJT - 1))

            den2 = P.tile([1, RB], F32, tag="den2")
            nc.scalar.activation(den2[:], numT2[F1:F1 + 1, 0:RB], AF.Copy)
            rec2 = P.tile([1, RB], F32, tag="rec2")
            nc.vector.reciprocal_approx_fast(rec2[:], den2[:])
            ps_r2 = PS.tile([128, 512], F32, tag="ps")
            nc.tensor.matmul(ps_r2[0:64, 0:RB], onesf1[:], rec2[:])
            rdb2 = GB.tile([128, RB], BF16, tag="rdb")
            nc.scalar.activation(rdb2[0:64, :], ps_r2[0:64, 0:RB], AF.Copy)

            # final divide+skip+store in i-halves so the first output DMA
            # overlaps the second half's compute
            for ih in range(2):
                sl = slice(ih * 256, (ih + 1) * 256)
                t2h = WK.tile([F1, 256], F32, tag="t2h")
                nc.vector.tensor_mul(t2h[:], numT2[0:F1, sl], rdb2[0:64, sl])
                o2h = WK.tile([F1, 256], F32, tag="o2h")
                nc.vector.scalar_tensor_tensor(
                    o2h[:], t2h[:], b2f[:], ps_sk2[0:64, sl], ALU.add, ALU.add)
                nc.sync.dma_start(d_out[:, sl], o2h[:])

    nc.compile()
    return nc


_NC_CACHE = None


def _get_nc():
    global _NC_CACHE
    if _NC_CACHE is None:
        _NC_CACHE = build_nc()
    return _NC_CACHE


def make_in_maps(x, adj, W1, a_src1, a_tgt1, Wskip1, b1, W2, a_src2, a_tgt2,
                 Wskip2, b2):
    x = np.asarray(x, np.float32)
    adj = np.asarray(adj, np.float32)
    W1 = np.asarray(W1, np.float32)
    W2 = np.asarray(W2, np.float32)
    Wskip1 = np.asarray(Wskip1, np.float32)
    Wskip2 = np.asarray(Wskip2, np.float32)
    w1Tb = np.ascontiguousarray(W1.T).astype(BF16NP)
    ws1Tb = np.ascontiguousarray(Wskip1.T).astype(BF16NP)
    w2Tb = np.ascontiguousarray(W2.T).astype(BF16NP)
    ws2Tb = np.ascontiguousarray(Wskip2.T).astype(BF16NP)
    a1s2 = np.ascontiguousarray(
        np.vstack([np.asarray(a_src1, np.float32).T] * 2))
    a1t2 = np.ascontiguousarray(
        np.vstack([np.asarray(a_tgt1, np.float32).T] * 2))
    a2p = np.ascontiguousarray(
        np.stack([np.asarray(a_src2, np.float32).ravel(),
                  np.asarray(a_tgt2, np.float32).ravel()], axis=1))
    xT = [np.ascontiguousarray(x[b].T) for b in range(BS)]
    xTb = [t.astype(BF16NP) for t in xT]
    adjTb = [np.ascontiguousarray(adj[b].T.astype(BF16NP)) for b in range(BS)]
    in_maps = []
    for c in range(NCORES):
        b, r = c // 4, c % 4
        sl = slice(r * RB, (r + 1) * RB)
        xrT = np.ascontiguousarray(xT[b][:, sl])
        in_maps.append({
            "adjT": np.ascontiguousarray(adjTb[b][:, sl]),
            "xT": xT[b], "xTb": xTb[b],
            "xrT": xrT, "xrTb": xrT.astype(BF16NP),
            "w1n": W1, "w1Tb": w1Tb, "ws1Tb": ws1Tb,
            "a1s2": a1s2, "a1t2": a1t2,
            "b1": np.asarray(b1, np.float32),
            "w2n": W2, "w2Tb": w2Tb, "ws2Tb": ws2Tb,
            "a2p": a2p,
            "b2": np.asarray(b2, np.float32),
        })
    return in_maps


def kernel(x, adj, W1, a_src1, a_tgt1, Wskip1, b1, W2, a_src2, a_tgt2,
           Wskip2, b2):
    nc = _get_nc()
    in_maps = make_in_maps(x, adj, W1, a_src1, a_tgt1, Wskip1, b1, W2,
                           a_src2, a_tgt2, Wskip2, b2)
    res = run_bass_kernel_spmd(nc, in_maps, core_ids=list(range(NCORES)))
    out = np.empty((BS, N, F1), np.float32)
    for c in range(NCORES):
        b, r = c // 4, c % 4
        out[b, r * RB:(r + 1) * RB, :] = res.results[c]["outT"].T
    return out
